# revision 9
# baseline (speedup 1.0000x reference)
"""Distributed Bass kernel for llama-style GQA attention on 8 trn2 NeuronCores.

Sharding: 2-way data-parallel over batch x 4-way tensor-parallel over heads.
Core c handles batch b=c//4 and head group t=c%4 (8 q-heads, 2 kv-heads).
wq/wk/wv split column-wise per head group; wo split row-wise; each core
produces a partial [S, HIDDEN] bf16 output, host sums the 4 partials per batch.

On-chip flow per core (all matmuls bf16, psum f32):
  xT (pre-transposed on host) @ wqkv -> k|q|v per 128-seq block; RoPE in
  planar layout (host pre-permutes wq/wk cols to [real|imag] halves); q,k
  flipped to [d, seq] via regular matmuls against a moving identity; v kept
  [seq, d] with appended ones cols (free softmax denominators).
  Attention pair rounds: heads (2t, 2t+1) share a kv head and live in
  opposite 64-partition halves of qT/kT, so their K=64 score matmuls
  auto-derive tile_position (0,0)/(64,0) and execute CONCURRENTLY on
  disjoint PE row-groups; one exp activation covers the pair's [128,1024]
  score psum (stale gap cols on diagonal rounds are exp'd but never read).
  The kernel is ACT(exp)-bound during attention (~1.1us exp vs ~0.75us PE
  per round), so proj/flip/wo matmul filler is spread across ALL attention:
    phase A : proj 0..13 + qb groups 0,1 (single-head units, baseline psum)
    phase A2: qb group 2 pair rounds + proj 14,15 (split in 4 pieces) +
              flips 12..15 + wo for seq blocks 0..3
    phase B : qb group 3 pair rounds (sT double-buffered, scores r+1
              emitted before ctx r) + wo for seq blocks 4..15 paced evenly
"""

import numpy as np
import ml_dtypes

import concourse.bass as bass
import concourse.mybir as mybir
import concourse.tile as tile
from concourse import bacc
from concourse.bass_utils import run_bass_kernel_spmd
from concourse.masks import make_identity

B, S, HID = 2, 2048, 2048
D = 64
NQ, NKV = 8, 2          # per-core heads
KW, QW, VW = NKV * D, NQ * D, NKV * D
QKVW = KW + QW + VW     # 768, layout [k(128) | q(512) | v(128)]
P = 128
SB = S // P             # 16 seq blocks
KC = HID // P           # 16 contraction chunks
F32 = mybir.dt.float32
BF16 = mybir.dt.bfloat16
BF = ml_dtypes.bfloat16
AF = mybir.ActivationFunctionType

_CACHE = {}


def _emit_graph(nc, tc, xT, wqkv, wo, cosb, sinb, out):
    with tc.tile_pool(name="const", bufs=1) as const, \
         tc.tile_pool(name="big", bufs=1) as big:
        # persistent tensors
        qT_sb = [big.tile([P, S], BF16, tag=f"qT{t}", name=f"qT{t}") for t in range(4)]
        kT_sb = [big.tile([P, S], BF16, tag=f"kT{k}", name=f"kT{k}") for k in range(NKV)]
        # vaug layout: per (sb, kv): 65 cols (64 v dims + 1 ones col)
        vaug_sb = big.tile([P, SB * NKV * 65], BF16, tag="va")
        ctxT_sb = [big.tile([P, S], BF16, tag=f"cT{t}", name=f"cT{t}") for t in range(4)]
        rot2 = big.tile([P, SB * 640], BF16, tag="rot2")   # roped k|q per sb
        wqkv_sb = big.tile([P, KC * QKVW], BF16, tag="wqkv")
        wo_sb = big.tile([P, 4 * HID], BF16, tag="wo")
        cos_sb = big.tile([P, SB * 32], BF16, tag="c1")
        sin_sb = big.tile([P, SB * 32], BF16, tag="s1")

        ident = const.tile([P, P], BF16, tag="id")
        make_identity(nc, ident[:, :])
        # tri01[k, q] = 1 where q >= k else 0 (keep-mask for aligned diag blocks)
        tri01 = const.tile([P, P], BF16, tag="tri")
        nc.gpsimd.memset(tri01[:, :], 1.0)
        nc.gpsimd.affine_select(
            out=tri01[:, :], in_=tri01[:, :], compare_op=mybir.AluOpType.is_ge,
            fill=0.0, base=0, pattern=[[1, P]], channel_multiplier=-1,
        )
        ones_cols = vaug_sb[:, :].rearrange("p (b c) -> p b c", c=65)[:, :, 64:65]
        nc.gpsimd.memset(ones_cols, 1.0)

        with tc.tile_pool(name="xp", bufs=2) as xp_p, \
             tc.tile_pool(name="rt", bufs=2) as rt_p, \
             tc.tile_pool(name="exs", bufs=6) as exs_p, \
             tc.tile_pool(name="nrm", bufs=6) as nrm_p, \
             tc.tile_pool(name="osb", bufs=6) as osb_p:

            xb_tiles = {}

            def xb_load(sb):
                xb = xp_p.tile([P, KC * P], BF16, tag="xb", name="xb")
                nc.sync.dma_start(out=xb[:, :], in_=xT[sb * P:(sb + 1) * P, :])
                xb_tiles[sb] = xb

            # first proj matmul gates only on xb chunk 0 + wqkv kc-block 0
            # (subtile deps), so load those first and stream the rest behind
            xb0 = xp_p.tile([P, KC * P], BF16, tag="xb", name="xb")
            nc.sync.dma_start(out=xb0[:, 0:512], in_=xT[0:P, 0:512])
            nc.sync.dma_start(out=wqkv_sb[:, 0:QKVW], in_=wqkv[:, 0:QKVW])
            nc.sync.dma_start(out=wqkv_sb[:, QKVW:4 * QKVW],
                              in_=wqkv[:, QKVW:4 * QKVW])
            nc.sync.dma_start(out=xb0[:, 512:2048], in_=xT[0:P, 512:2048])
            for c in range(1, 4):
                nc.sync.dma_start(out=wqkv_sb[:, c * 4 * QKVW:(c + 1) * 4 * QKVW],
                                  in_=wqkv[:, c * 4 * QKVW:(c + 1) * 4 * QKVW])
            xb_tiles[0] = xb0
            nc.sync.dma_start(out=cos_sb[:, :], in_=cosb)
            nc.sync.dma_start(out=sin_sb[:, :], in_=sinb)

            def rope_and_v(sb, pga, pgb):
                rsrc = rt_p.tile([P, KW + QW], BF16, tag="rsrc", name="rsrc")
                nc.vector.tensor_copy(rsrc[:, 0:512], pga[:, :])
                nc.vector.tensor_copy(rsrc[:, 512:KW + QW], pgb[:, 0:128])
                nh = NKV + NQ
                rs = rsrc[:, :].rearrange("p (h i) -> p h i", h=nh)
                ev, od = rs[:, :, 0:32], rs[:, :, 32:64]
                c = cos_sb[:, sb * 32:(sb + 1) * 32].rearrange("p (o i) -> p o i", o=1).broadcast_to((P, nh, 32))
                s = sin_sb[:, sb * 32:(sb + 1) * 32].rearrange("p (o i) -> p o i", o=1).broadcast_to((P, nh, 32))
                t1 = rt_p.tile([P, nh * 32], BF16, tag="t1", name="t1")
                t2 = rt_p.tile([P, nh * 32], BF16, tag="t2", name="t2")
                t1r = t1[:, :].rearrange("p (h i) -> p h i", h=nh)
                t2r = t2[:, :].rearrange("p (h i) -> p h i", h=nh)
                dst = rot2[:, sb * 640:(sb + 1) * 640].rearrange("p (h i) -> p h i", h=nh)
                dst_e, dst_o = dst[:, :, 0:32], dst[:, :, 32:64]
                nc.vector.tensor_mul(t1r, ev, c)
                nc.vector.tensor_mul(t2r, od, s)
                nc.vector.tensor_sub(dst_e, t1r, t2r)
                nc.vector.tensor_mul(t1r, ev, s)
                nc.vector.tensor_mul(t2r, od, c)
                nc.vector.tensor_add(dst_o, t1r, t2r)
                vdst = vaug_sb[:, sb * 130: sb * 130 + 130].rearrange("p (kv c) -> p kv c", kv=2)[:, :, 0:D]
                vsrc = pgb[:, 128:256].rearrange("p (kv c) -> p kv c", kv=2)
                nc.vector.tensor_copy(vdst, vsrc)

            def wo_unit(pso_p, sb, n, copy_eng="v", tag="po"):
                po = pso_p.tile([P, 512], F32, tag=tag, name="po")
                for c in range(4):
                    nc.tensor.matmul(po[:, :], ctxT_sb[c][:, sb * P:(sb + 1) * P],
                                     wo_sb[:, c * HID + n * 512: c * HID + (n + 1) * 512],
                                     start=(c == 0), stop=(c == 3))
                ob = osb_p.tile([P, 512], BF16, tag="ob", name="ob")
                if copy_eng == "s":
                    nc.scalar.activation(ob[:, :], po[:, :], AF.Copy)
                else:
                    nc.vector.tensor_copy(ob[:, :], po[:, :])
                eng = nc.gpsimd if (sb + n) % 2 else nc.sync
                eng.dma_start(out=out[sb * P:(sb + 1) * P, n * 512:(n + 1) * 512],
                              in_=ob[:, :])

            def normalize(ctx, t, roff, qb):
                # copy ctx out of psum immediately (den + staged numerator)
                # so the pair's ctx psum frees after ~1us instead of after
                # the whole broadcast/recip/mul chain - the next unit's
                # first ctx matmul WAR-waits on this release
                den = nrm_p.tile([1, 512], F32, tag="den", name="den")
                nc.vector.tensor_copy(den[:, :], ctx[64:65, :])
                cstg = nrm_p.tile([D, 512], BF16, tag="cstg", name="cstg")
                nc.vector.tensor_copy(cstg[:, :], ctx[0:D, :])
                dbc = nrm_p.tile([D, 512], F32, tag="dbc", name="dbc")
                nc.gpsimd.partition_broadcast(dbc[:, :], den[:, :])
                rbc = nrm_p.tile([D, 512], F32, tag="rbc", name="rbc")
                nc.vector.reciprocal_approx_fast(out=rbc[:, :], in_=dbc[:, :])
                ntmp = nrm_p.tile([D, 512], BF16, tag="ntmp", name="ntmp")
                # qb=3 in two chunks: the tail wo units for sb 12,13 gate on
                # the first half of every head's last ctxT block
                cw = (256,) if qb == 3 else (512,)
                c0 = 0
                for w in (cw * 2 if qb == 3 else cw):
                    nc.vector.tensor_mul(ntmp[:, c0:c0 + w], cstg[:, c0:c0 + w], rbc[:, c0:c0 + w])
                    nc.sync.dma_start(
                        out=ctxT_sb[t][roff:roff + D, qb * 512 + c0:qb * 512 + c0 + w],
                        in_=ntmp[:, c0:c0 + w])
                    c0 += w

            def flip_closures(pt_pool, pt_tag, sb):
                # q/k flips: regular matmul with the block as the stationary
                # operand and identity as the moving one (out = blk.T @ I).
                def t_k():
                    pt = pt_pool.tile([P, P], F32, tag=pt_tag, name="pt")
                    nc.tensor.matmul(pt[:, :], rot2[:, sb * 640: sb * 640 + P],
                                     ident[:, :], start=True, stop=True)
                    nc.vector.tensor_copy(kT_sb[0][0:D, sb * P:(sb + 1) * P], pt[0:D, :])
                    nc.vector.tensor_copy(kT_sb[1][D:P, sb * P:(sb + 1) * P], pt[D:P, :])
                    nc.sync.dma_start(out=kT_sb[0][D:P, sb * P:(sb + 1) * P],
                                      in_=kT_sb[0][0:D, sb * P:(sb + 1) * P])
                    nc.sync.dma_start(out=kT_sb[1][0:D, sb * P:(sb + 1) * P],
                                      in_=kT_sb[1][D:P, sb * P:(sb + 1) * P])

                def t_q(t):
                    def f():
                        pt = pt_pool.tile([P, P], F32, tag=pt_tag, name="pt")
                        nc.tensor.matmul(pt[:, :],
                                         rot2[:, sb * 640 + KW + t * P: sb * 640 + KW + (t + 1) * P],
                                         ident[:, :], start=True, stop=True)
                        nc.vector.tensor_copy(qT_sb[t][:, sb * P:(sb + 1) * P], pt[:, :])
                    return f
                return [t_k] + [t_q(t) for t in range(4)]

            def proj_mms(psg_p, sb, kc_lo, kc_hi, tq, ti_box):
                """One piece of the qkv projection for seq block sb."""
                if kc_lo == 0:
                    if sb not in xb_tiles:
                        xb_load(sb)
                    if sb + 1 <= 15 and sb + 1 not in xb_tiles:
                        xb_load(sb + 1)   # prefetch next block
                    pga = psg_p.tile([P, 512], F32, tag="pga", name="pga")
                    pgb = psg_p.tile([P, 256], F32, tag="pgb", name="pgb")
                    xb_tiles[(sb, "ps")] = (pga, pgb)
                pga, pgb = xb_tiles[(sb, "ps")]
                xb = xb_tiles[sb]
                for kc in range(kc_lo, kc_hi):
                    nc.tensor.matmul(pga[:, :], xb[:, kc * P:(kc + 1) * P],
                                     wqkv_sb[:, kc * QKVW:kc * QKVW + 512],
                                     start=(kc == 0), stop=(kc == KC - 1))
                    nc.tensor.matmul(pgb[:, :], xb[:, kc * P:(kc + 1) * P],
                                     wqkv_sb[:, kc * QKVW + 512:(kc + 1) * QKVW],
                                     start=(kc == 0), stop=(kc == KC - 1))
                    if ti_box[0] < len(tq) and kc in (3, 6, 9, 12, 15):
                        tq[ti_box[0]]()
                        ti_box[0] += 1
                if kc_hi == KC:
                    while ti_box[0] < len(tq):
                        tq[ti_box[0]]()
                        ti_box[0] += 1
                    # stream in a quarter of wo during phase A (needed from
                    # A2 on; too big to load in one go at that point)
                    if sb in (2, 4, 6, 8):
                        c = (sb - 2) // 2
                        nc.sync.dma_start(out=wo_sb[:, c * HID:(c + 1) * HID],
                                          in_=wo[:, c * HID:(c + 1) * HID])
                    rope_and_v(sb, pga, pgb)
                    del xb_tiles[(sb, "ps")]
                    del xb_tiles[sb]

            def proj(psg_p, sb, tq=()):
                proj_mms(psg_p, sb, 0, KC, tq, [0])

            def emit_scores(ps_p, t, qb, kb):
                # head pair (2t, 2t+1): concurrent K=64 matmuls on row
                # groups (0,0)/(64,0); outputs in the two banks of one
                # [128,1024] f32 tile; one exp covers both halves (incl.
                # the stale gap cols on diagonal rounds - never read back)
                kv = t // 2
                j0 = max(kb - 4 * qb, 0)
                sT = ps_p.tile([P, 1024], F32, tag="sT", name="sT")
                nc.tensor.matmul(sT[:, j0 * P:512],
                                 kT_sb[kv][0:D, kb * P:(kb + 1) * P],
                                 qT_sb[t][0:D, qb * 512 + j0 * P:(qb + 1) * 512],
                                 start=True, stop=True)
                nc.tensor.matmul(sT[:, 512 + j0 * P:1024],
                                 kT_sb[kv][D:P, kb * P:(kb + 1) * P],
                                 qT_sb[t][D:P, qb * 512 + j0 * P:(qb + 1) * 512],
                                 start=True, stop=True)
                ex = exs_p.tile([P, 1024], BF16, tag="ex2", name="ex2")
                nc.scalar.activation(ex[:, j0 * P:1024], sT[:, j0 * P:1024],
                                     AF.Exp, scale=0.125)
                return ex

            def emit_ctx(ctxA, ctxB, t, qb, kb, nkb, ex):
                kv = t // 2
                j0 = max(kb - 4 * qb, 0)
                if kb >= 4 * qb:
                    nc.vector.tensor_mul(ex[:, j0 * P:(j0 + 1) * P],
                                         ex[:, j0 * P:(j0 + 1) * P], tri01[:, :])
                    nc.vector.tensor_mul(ex[:, 512 + j0 * P:512 + (j0 + 1) * P],
                                         ex[:, 512 + j0 * P:512 + (j0 + 1) * P], tri01[:, :])
                va = vaug_sb[:, kb * 130 + kv * 65: kb * 130 + (kv + 1) * 65]
                nc.tensor.matmul(ctxA[:, j0 * P:512], va, ex[:, j0 * P:512],
                                 start=(kb == 0), stop=(kb == nkb - 1),
                                 skip_group_check=True)
                nc.tensor.matmul(ctxB[:, j0 * P:512], va, ex[:, 512 + j0 * P:1024],
                                 start=(kb == 0), stop=(kb == nkb - 1),
                                 skip_group_check=True)
                if kb == nkb - 1:
                    normalize(ctxA, t, 0, qb)
                    normalize(ctxB, t, D, qb)

            # ======== phase A: proj 0..13 + attention groups 0,1 ========
            with tc.tile_pool(name="psg", bufs=2, space="PSUM") as psg_p, \
                 tc.tile_pool(name="pst", bufs=1, space="PSUM") as pst_p, \
                 tc.tile_pool(name="pssA", bufs=2, space="PSUM") as pssA_p, \
                 tc.tile_pool(name="pscA", bufs=1, space="PSUM") as pscA_p:

                def attn_unit_a(h, qb):
                    t, roff, kv = h // 2, D * (h % 2), h // 4
                    qT = qT_sb[t][roff:roff + D, :]
                    kT = kT_sb[kv][roff:roff + D, :]
                    ctx = pscA_p.tile([65, 512], F32, tag="ctx", name="ctx")
                    nkb = 4 * qb + 4
                    for kb in range(nkb):
                        j0 = max(kb - 4 * qb, 0)
                        sT = pssA_p.tile([P, 512], F32, tag="sT", name="sT")
                        nc.tensor.matmul(sT[:, j0 * P:512], kT[:, kb * P:(kb + 1) * P],
                                         qT[:, qb * 512 + j0 * P:(qb + 1) * 512], start=True, stop=True)
                        ex = exs_p.tile([P, 512], BF16, tag="ex", name="ex")
                        nc.scalar.activation(ex[:, j0 * P:512], sT[:, j0 * P:512], AF.Exp, scale=0.125)
                        if kb >= 4 * qb:
                            nc.vector.tensor_mul(ex[:, j0 * P:(j0 + 1) * P],
                                                 ex[:, j0 * P:(j0 + 1) * P], tri01[:, :])
                        nc.tensor.matmul(ctx[:, j0 * P:512],
                                         vaug_sb[:, kb * 130 + kv * 65: kb * 130 + (kv + 1) * 65],
                                         ex[:, j0 * P:512], start=(kb == 0), stop=(kb == nkb - 1),
                                         skip_group_check=True)
                    normalize(ctx, t, roff, qb)

                fa = lambda sb: flip_closures(pst_p, "pt", sb)
                proj(psg_p, 0)
                proj(psg_p, 1)
                for sb in (2, 3, 4, 5):
                    proj(psg_p, sb, fa(sb - 2))
                # group 0 (qb=0) with proj 6..9, two units per proj
                proj(psg_p, 6, fa(4))
                attn_unit_a(0, 0)
                attn_unit_a(1, 0)
                proj(psg_p, 7, fa(5))
                attn_unit_a(2, 0)
                attn_unit_a(3, 0)
                proj(psg_p, 8, fa(6))
                attn_unit_a(4, 0)
                attn_unit_a(5, 0)
                proj(psg_p, 9, fa(7))
                attn_unit_a(6, 0)
                attn_unit_a(7, 0)
                # group 1 (qb=1) with proj 10..13, two units per proj
                proj(psg_p, 10, fa(8))
                attn_unit_a(0, 1)
                attn_unit_a(1, 1)
                proj(psg_p, 11, fa(9))
                attn_unit_a(2, 1)
                attn_unit_a(3, 1)
                proj(psg_p, 12, fa(10))
                attn_unit_a(4, 1)
                attn_unit_a(5, 1)
                proj(psg_p, 13, fa(11))
                attn_unit_a(6, 1)
                attn_unit_a(7, 1)

            # ======== phase A2: qb group 2 pair rounds + proj 14,15 ========
            # + flips 12..15 + wo for seq blocks 0..3
            with tc.tile_pool(name="psg2", bufs=1, space="PSUM") as psg2_p, \
                 tc.tile_pool(name="ps2a", bufs=1, space="PSUM") as ps2a_p, \
                 tc.tile_pool(name="psca", bufs=1, space="PSUM") as psca_p, \
                 tc.tile_pool(name="psoa", bufs=2, space="PSUM") as psoa_p:

                # filler items, emitted between attention rounds
                items = []
                ti14, ti15 = [0], [0]
                tq14 = flip_closures(psoa_p, "po", 12)
                tq15 = flip_closures(psoa_p, "po", 13)
                for piece in range(4):
                    items.append(lambda p=piece: proj_mms(psg2_p, 14, p * 4, (p + 1) * 4, tq14, ti14))
                for piece in range(4):
                    items.append(lambda p=piece: proj_mms(psg2_p, 15, p * 4, (p + 1) * 4, tq15, ti15))
                for f in flip_closures(psoa_p, "po", 14):
                    items.append(f)
                for f in flip_closures(psoa_p, "po", 15):
                    items.append(f)
                for sb in (0, 1, 2, 3):
                    for n in range(4):
                        items.append(lambda sb=sb, n=n: wo_unit(psoa_p, sb, n))

                rounds = [(t, 2, kb, 12) for t in range(4) for kb in range(12)]
                ctx_tiles = {}

                def get_ctx(pool, kb):
                    if kb == 0:
                        ctx_tiles[0] = pool.tile([65, 512], F32, tag="ctxA", name="ctxA")
                        ctx_tiles[1] = pool.tile([65, 512], F32, tag="ctxB", name="ctxB")
                    return ctx_tiles[0], ctx_tiles[1]

                ii = 0
                n_spread = len(items) - 4   # last 4 bridge the A2->B seam
                pend = None
                for ri, (t, qb, kb, nkb) in enumerate(rounds):
                    ex = emit_scores(ps2a_p, t, qb, kb)
                    while ii < n_spread and ii < (ri + 1) * n_spread // len(rounds):
                        items[ii]()
                        ii += 1
                    if pend is not None:
                        (pt_, pqb, pkb, pnkb, pex) = pend
                        ctxA, ctxB = get_ctx(psca_p, pkb)
                        emit_ctx(ctxA, ctxB, pt_, pqb, pkb, pnkb, pex)
                    pend = (t, qb, kb, nkb, ex)
                if pend is not None:
                    (pt_, pqb, pkb, pnkb, pex) = pend
                    ctxA, ctxB = get_ctx(psca_p, pkb)
                    emit_ctx(ctxA, ctxB, pt_, pqb, pkb, pnkb, pex)
                while ii < len(items):
                    items[ii]()
                    ii += 1

            # ======== phase B: qb group 3 pair rounds + wo 4..15 ========
            with tc.tile_pool(name="ps2", bufs=2, space="PSUM") as ps2_p, \
                 tc.tile_pool(name="psc", bufs=1, space="PSUM") as psc_p, \
                 tc.tile_pool(name="pso", bufs=2, space="PSUM") as pso_p:

                wos = [(sb, n) for sb in range(4, 16) for n in range(4)]
                wi = 0

                rounds = [(t, 3, kb, 16) for t in range(4) for kb in range(16)]
                ctx_tiles2 = {}

                def get_ctx2(kb):
                    if kb == 0:
                        ctx_tiles2[0] = psc_p.tile([65, 512], F32, tag="ctxA", name="ctxA")
                        ctx_tiles2[1] = psc_p.tile([65, 512], F32, tag="ctxB", name="ctxB")
                    return ctx_tiles2[0], ctx_tiles2[1]

                # software pipeline: scores r+1 before ctx r; wo for sb<12
                # (ready since A2) paced ~1 per 2 rounds, holding 2 back to
                # cover the rounds->tail boundary; sb>=12 gates on qb3 ctxT
                # and runs in the tail over rotated psum slots.
                pend = None
                for ri, (t, qb, kb, nkb) in enumerate(rounds):
                    ex = emit_scores(ps2_p, t, qb, kb)
                    if wi < 26 and ri % 2 == 0:
                        wo_unit(pso_p, *wos[wi])
                        wi += 1
                    if pend is not None:
                        (pt_, pqb, pkb, pnkb, pex) = pend
                        ctxA, ctxB = get_ctx2(pkb)
                        emit_ctx(ctxA, ctxB, pt_, pqb, pkb, pnkb, pex)
                    pend = (t, qb, kb, nkb, ex)
                if pend is not None:
                    (pt_, pqb, pkb, pnkb, pex) = pend
                    ctxA, ctxB = get_ctx2(pkb)
                    emit_ctx(ctxA, ctxB, pt_, pqb, pkb, pnkb, pex)
                # tail: the 2 held-back units bridge the normalize latency,
                # then sb 12..15 over rotated psum slots, copies alternating
                # between the idle scalar and vector engines
                tail_slots = [(pso_p, "po"), (psc_p, "ctxA"), (ps2_p, "sT"),
                              (pso_p, "po"), (psc_p, "ctxB"), (ps2_p, "sT")]
                for i, u in enumerate(wos[wi:]):
                    pool, tag = tail_slots[i % len(tail_slots)]
                    wo_unit(pool, *u, copy_eng=("s" if i % 2 else "v"), tag=tag)


def _build():
    nc = bacc.Bacc("TRN2", target_bir_lowering=False, debug=False, num_devices=8)
    # all inputs host-pre-permuted into DMA-friendly layouts (fat packets):
    # xT[sb*P+p, kc*P+j] = x.T[kc*P+p, sb*P+j]; weights/tables partition-major
    xT = nc.dram_tensor("xT", [S, HID], BF16, kind="ExternalInput").ap()
    wqkv = nc.dram_tensor("wqkv", [P, KC * QKVW], BF16, kind="ExternalInput").ap()
    wo = nc.dram_tensor("wo", [P, 4 * HID], BF16, kind="ExternalInput").ap()
    cosb = nc.dram_tensor("cosb", [P, SB * 32], BF16, kind="ExternalInput").ap()
    sinb = nc.dram_tensor("sinb", [P, SB * 32], BF16, kind="ExternalInput").ap()
    out = nc.dram_tensor("out", [S, HID], BF16, kind="ExternalOutput").ap()
    with tile.TileContext(nc) as tc:
        _emit_graph(nc, tc, xT, wqkv, wo, cosb, sinb, out)
    nc.finalize()
    return nc


def _planar(w, nheads):
    """Permute each head's 64 cols from interleaved (r0,i0,r1,i1,...) to
    planar (r0..r31, i0..i31)."""
    h = w.reshape(w.shape[0], nheads, 32, 2)
    return np.ascontiguousarray(h.transpose(0, 1, 3, 2).reshape(w.shape[0], nheads * 64))


def kernel(x, wq, wk, wv, wo, freqs_cos, freqs_sin, mask):
    x = np.asarray(x, dtype=np.float32)
    wq = np.asarray(wq, dtype=np.float32)
    wk = np.asarray(wk, dtype=np.float32)
    wv = np.asarray(wv, dtype=np.float32)
    wo = np.asarray(wo, dtype=np.float32)
    fc = np.asarray(freqs_cos, dtype=np.float32)
    fs = np.asarray(freqs_sin, dtype=np.float32)

    if "nc" not in _CACHE:
        _CACHE["nc"] = _build()
    nc = _CACHE["nc"]

    wqp = _planar(wq, 32)   # planar per q head
    wkp = _planar(wk, 8)    # planar per kv head
    cosb = np.ascontiguousarray(fc.reshape(SB, P, 32).transpose(1, 0, 2).reshape(P, SB * 32)).astype(BF)
    sinb = np.ascontiguousarray(fs.reshape(SB, P, 32).transpose(1, 0, 2).reshape(P, SB * 32)).astype(BF)
    in_maps = []
    for core in range(8):
        b, t = core // 4, core % 4
        # xT[sb*P+p, kc*P+j] = x[b][sb*P+j, kc*P+p]
        xTb = x[b].reshape(SB, P, KC, P).transpose(0, 3, 2, 1).reshape(S, HID)
        wqkv_full = np.concatenate(
            [wkp[:, t * KW:(t + 1) * KW],
             wqp[:, t * QW:(t + 1) * QW],
             wv[:, t * VW:(t + 1) * VW]], axis=1)   # [HID, 768]
        wqkv_p = wqkv_full.reshape(KC, P, QKVW).transpose(1, 0, 2).reshape(P, KC * QKVW)
        wo_p = wo[t * QW:(t + 1) * QW, :].reshape(4, P, HID).transpose(1, 0, 2).reshape(P, 4 * HID)
        in_maps.append({
            "xT": np.ascontiguousarray(xTb).astype(BF),
            "wqkv": np.ascontiguousarray(wqkv_p).astype(BF),
            "wo": np.ascontiguousarray(wo_p).astype(BF),
            "cosb": cosb, "sinb": sinb,
        })
    trace = bool(_CACHE.get("trace"))
    try:
        res = run_bass_kernel_spmd(nc, in_maps, list(range(8)), trace=trace)
    except Exception:
        if not trace:
            raise
        res = run_bass_kernel_spmd(nc, in_maps, list(range(8)))
    _CACHE["last_result"] = res
    outs = [np.asarray(r["out"], dtype=np.float32) for r in res.results]
    full = np.stack([outs[0] + outs[1] + outs[2] + outs[3],
                     outs[4] + outs[5] + outs[6] + outs[7]], axis=0)
    return full


# revision 10
# speedup vs baseline: 1.0129x; 1.0129x over previous
"""Distributed Bass kernel for llama-style GQA attention on 8 trn2 NeuronCores.

Sharding: 2-way data-parallel over batch x 4-way tensor-parallel over heads.
Core c handles batch b=c//4 and head group t=c%4 (8 q-heads, 2 kv-heads).
wq/wk/wv split column-wise per head group; wo split row-wise; each core
produces a partial [S, HIDDEN] bf16 output, host sums the 4 partials per batch.

On-chip flow per core (all matmuls bf16, psum f32):
  xT (pre-transposed on host) @ wqkv -> k|q|v per 128-seq block; RoPE in
  planar layout (host pre-permutes wq/wk cols to [real|imag] halves); q,k
  flipped to [d, seq] via regular matmuls against a moving identity; v kept
  [seq, d] with appended ones cols (free softmax denominators).
  Attention pair rounds: heads (2t, 2t+1) share a kv head and live in
  opposite 64-partition halves of qT/kT, so their K=64 score matmuls
  auto-derive tile_position (0,0)/(64,0) and execute CONCURRENTLY on
  disjoint PE row-groups; one exp activation covers the pair's [128,1024]
  score psum (stale gap cols on diagonal rounds are exp'd but never read).
  The kernel is ACT(exp)-bound during attention (~1.1us exp vs ~0.75us PE
  per round), so proj/flip/wo matmul filler is spread across ALL attention:
    phase A : proj 0..13 + qb groups 0,1 (single-head units, baseline psum)
    phase A2: qb group 2 pair rounds + proj 14,15 (split in 4 pieces) +
              flips 12..15 + wo for seq blocks 0..3
    phase B : qb group 3 pair rounds (sT double-buffered, scores r+1
              emitted before ctx r) + wo for seq blocks 4..15 paced evenly
"""

import numpy as np
import ml_dtypes

import concourse.bass as bass
import concourse.mybir as mybir
import concourse.tile as tile
from concourse import bacc
from concourse.bass_utils import run_bass_kernel_spmd
from concourse.masks import make_identity

B, S, HID = 2, 2048, 2048
D = 64
NQ, NKV = 8, 2          # per-core heads
KW, QW, VW = NKV * D, NQ * D, NKV * D
QKVW = KW + QW + VW     # 768, layout [k(128) | q(512) | v(128)]
P = 128
SB = S // P             # 16 seq blocks
KC = HID // P           # 16 contraction chunks
F32 = mybir.dt.float32
BF16 = mybir.dt.bfloat16
BF = ml_dtypes.bfloat16
AF = mybir.ActivationFunctionType

_CACHE = {}


def _emit_graph(nc, tc, xT, wqkv, wo, cosb, sinb, out):
    with tc.tile_pool(name="const", bufs=1) as const, \
         tc.tile_pool(name="big", bufs=1) as big:
        # persistent tensors
        qT_sb = [big.tile([P, S], BF16, tag=f"qT{t}", name=f"qT{t}") for t in range(4)]
        kT_sb = [big.tile([P, S], BF16, tag=f"kT{k}", name=f"kT{k}") for k in range(NKV)]
        # vaug layout: per (sb, kv): 65 cols (64 v dims + 1 ones col)
        vaug_sb = big.tile([P, SB * NKV * 65], BF16, tag="va")
        ctxT_sb = [big.tile([P, S], BF16, tag=f"cT{t}", name=f"cT{t}") for t in range(4)]
        rot2 = big.tile([P, SB * 640], BF16, tag="rot2")   # roped k|q per sb
        wqkv_sb = big.tile([P, KC * QKVW], BF16, tag="wqkv")
        wo_sb = big.tile([P, 4 * HID], BF16, tag="wo")
        cos_sb = big.tile([P, SB * 32], BF16, tag="c1")
        sin_sb = big.tile([P, SB * 32], BF16, tag="s1")

        ident = const.tile([P, P], BF16, tag="id")
        make_identity(nc, ident[:, :])
        # tri01[k, q] = 1 where q >= k else 0 (keep-mask for aligned diag blocks)
        tri01 = const.tile([P, P], BF16, tag="tri")
        nc.gpsimd.memset(tri01[:, :], 1.0)
        nc.gpsimd.affine_select(
            out=tri01[:, :], in_=tri01[:, :], compare_op=mybir.AluOpType.is_ge,
            fill=0.0, base=0, pattern=[[1, P]], channel_multiplier=-1,
        )
        ones_cols = vaug_sb[:, :].rearrange("p (b c) -> p b c", c=65)[:, :, 64:65]
        nc.gpsimd.memset(ones_cols, 1.0)

        with tc.tile_pool(name="xp", bufs=2) as xp_p, \
             tc.tile_pool(name="rt", bufs=2) as rt_p, \
             tc.tile_pool(name="exs", bufs=6) as exs_p, \
             tc.tile_pool(name="nrm", bufs=6) as nrm_p, \
             tc.tile_pool(name="osb", bufs=6) as osb_p:

            xb_tiles = {}

            def xb_load(sb):
                xb = xp_p.tile([P, KC * P], BF16, tag="xb", name="xb")
                nc.sync.dma_start(out=xb[:, :], in_=xT[sb * P:(sb + 1) * P, :])
                xb_tiles[sb] = xb

            # first proj matmul gates only on xb chunk 0 + wqkv kc-block 0
            # (subtile deps), so load those first and stream the rest behind
            xb0 = xp_p.tile([P, KC * P], BF16, tag="xb", name="xb")
            nc.sync.dma_start(out=xb0[:, 0:512], in_=xT[0:P, 0:512])
            nc.sync.dma_start(out=wqkv_sb[:, 0:QKVW], in_=wqkv[:, 0:QKVW])
            nc.sync.dma_start(out=wqkv_sb[:, QKVW:4 * QKVW],
                              in_=wqkv[:, QKVW:4 * QKVW])
            nc.sync.dma_start(out=xb0[:, 512:2048], in_=xT[0:P, 512:2048])
            for c in range(1, 4):
                nc.sync.dma_start(out=wqkv_sb[:, c * 4 * QKVW:(c + 1) * 4 * QKVW],
                                  in_=wqkv[:, c * 4 * QKVW:(c + 1) * 4 * QKVW])
            xb_tiles[0] = xb0
            nc.sync.dma_start(out=cos_sb[:, :], in_=cosb)
            nc.sync.dma_start(out=sin_sb[:, :], in_=sinb)

            def rope_and_v(sb, pga, pgb):
                rsrc = rt_p.tile([P, KW + QW], BF16, tag="rsrc", name="rsrc")
                nc.vector.tensor_copy(rsrc[:, 0:512], pga[:, :])
                nc.vector.tensor_copy(rsrc[:, 512:KW + QW], pgb[:, 0:128])
                nh = NKV + NQ
                rs = rsrc[:, :].rearrange("p (h i) -> p h i", h=nh)
                ev, od = rs[:, :, 0:32], rs[:, :, 32:64]
                c = cos_sb[:, sb * 32:(sb + 1) * 32].rearrange("p (o i) -> p o i", o=1).broadcast_to((P, nh, 32))
                s = sin_sb[:, sb * 32:(sb + 1) * 32].rearrange("p (o i) -> p o i", o=1).broadcast_to((P, nh, 32))
                t1 = rt_p.tile([P, nh * 32], BF16, tag="t1", name="t1")
                t2 = rt_p.tile([P, nh * 32], BF16, tag="t2", name="t2")
                t1r = t1[:, :].rearrange("p (h i) -> p h i", h=nh)
                t2r = t2[:, :].rearrange("p (h i) -> p h i", h=nh)
                dst = rot2[:, sb * 640:(sb + 1) * 640].rearrange("p (h i) -> p h i", h=nh)
                dst_e, dst_o = dst[:, :, 0:32], dst[:, :, 32:64]
                nc.vector.tensor_mul(t1r, ev, c)
                nc.vector.tensor_mul(t2r, od, s)
                nc.vector.tensor_sub(dst_e, t1r, t2r)
                nc.vector.tensor_mul(t1r, ev, s)
                nc.vector.tensor_mul(t2r, od, c)
                nc.vector.tensor_add(dst_o, t1r, t2r)
                vdst = vaug_sb[:, sb * 130: sb * 130 + 130].rearrange("p (kv c) -> p kv c", kv=2)[:, :, 0:D]
                vsrc = pgb[:, 128:256].rearrange("p (kv c) -> p kv c", kv=2)
                nc.vector.tensor_copy(vdst, vsrc)

            def wo_unit(pso_p, sb, n, copy_eng="v", tag="po"):
                po = pso_p.tile([P, 512], F32, tag=tag, name="po")
                for c in range(4):
                    nc.tensor.matmul(po[:, :], ctxT_sb[c][:, sb * P:(sb + 1) * P],
                                     wo_sb[:, c * HID + n * 512: c * HID + (n + 1) * 512],
                                     start=(c == 0), stop=(c == 3))
                ob = osb_p.tile([P, 512], BF16, tag="ob", name="ob")
                if copy_eng == "s":
                    nc.scalar.activation(ob[:, :], po[:, :], AF.Copy)
                else:
                    nc.vector.tensor_copy(ob[:, :], po[:, :])
                eng = nc.gpsimd if (sb + n) % 2 else nc.sync
                eng.dma_start(out=out[sb * P:(sb + 1) * P, n * 512:(n + 1) * 512],
                              in_=ob[:, :])

            def normalize(ctx, t, roff, qb):
                # copy ctx out of psum immediately (den + staged numerator)
                # so the pair's ctx psum frees after ~1us instead of after
                # the whole broadcast/recip/mul chain - the next unit's
                # first ctx matmul WAR-waits on this release
                den = nrm_p.tile([1, 512], F32, tag="den", name="den")
                nc.vector.tensor_copy(den[:, :], ctx[64:65, :])
                cstg = nrm_p.tile([D, 512], BF16, tag="cstg", name="cstg")
                nc.vector.tensor_copy(cstg[:, :], ctx[0:D, :])
                dbc = nrm_p.tile([D, 512], F32, tag="dbc", name="dbc")
                nc.gpsimd.partition_broadcast(dbc[:, :], den[:, :])
                rbc = nrm_p.tile([D, 512], F32, tag="rbc", name="rbc")
                nc.vector.reciprocal_approx_fast(out=rbc[:, :], in_=dbc[:, :])
                ntmp = nrm_p.tile([D, 512], BF16, tag="ntmp", name="ntmp")
                # qb=3 in two chunks: the tail wo units for sb 12,13 gate on
                # the first half of every head's last ctxT block
                cw = (256,) if qb == 3 else (512,)
                c0 = 0
                for w in (cw * 2 if qb == 3 else cw):
                    nc.vector.tensor_mul(ntmp[:, c0:c0 + w], cstg[:, c0:c0 + w], rbc[:, c0:c0 + w])
                    nc.sync.dma_start(
                        out=ctxT_sb[t][roff:roff + D, qb * 512 + c0:qb * 512 + c0 + w],
                        in_=ntmp[:, c0:c0 + w])
                    c0 += w

            def flip_closures(pt_pool, pt_tag, sb):
                # q/k flips: regular matmul with the block as the stationary
                # operand and identity as the moving one (out = blk.T @ I).
                def t_k():
                    pt = pt_pool.tile([P, P], F32, tag=pt_tag, name="pt")
                    nc.tensor.matmul(pt[:, :], rot2[:, sb * 640: sb * 640 + P],
                                     ident[:, :], start=True, stop=True)
                    nc.vector.tensor_copy(kT_sb[0][0:D, sb * P:(sb + 1) * P], pt[0:D, :])
                    nc.vector.tensor_copy(kT_sb[1][D:P, sb * P:(sb + 1) * P], pt[D:P, :])
                    nc.sync.dma_start(out=kT_sb[0][D:P, sb * P:(sb + 1) * P],
                                      in_=kT_sb[0][0:D, sb * P:(sb + 1) * P])
                    nc.sync.dma_start(out=kT_sb[1][0:D, sb * P:(sb + 1) * P],
                                      in_=kT_sb[1][D:P, sb * P:(sb + 1) * P])

                def t_q(t):
                    def f():
                        pt = pt_pool.tile([P, P], F32, tag=pt_tag, name="pt")
                        nc.tensor.matmul(pt[:, :],
                                         rot2[:, sb * 640 + KW + t * P: sb * 640 + KW + (t + 1) * P],
                                         ident[:, :], start=True, stop=True)
                        nc.vector.tensor_copy(qT_sb[t][:, sb * P:(sb + 1) * P], pt[:, :])
                    return f
                return [t_k] + [t_q(t) for t in range(4)]

            def proj_mms(psg_p, sb, kc_lo, kc_hi, tq, ti_box):
                """One piece of the qkv projection for seq block sb."""
                if kc_lo == 0:
                    if sb not in xb_tiles:
                        xb_load(sb)
                    if sb + 1 <= 15 and sb + 1 not in xb_tiles:
                        xb_load(sb + 1)   # prefetch next block
                    pga = psg_p.tile([P, 512], F32, tag="pga", name="pga")
                    pgb = psg_p.tile([P, 256], F32, tag="pgb", name="pgb")
                    xb_tiles[(sb, "ps")] = (pga, pgb)
                pga, pgb = xb_tiles[(sb, "ps")]
                xb = xb_tiles[sb]
                for kc in range(kc_lo, kc_hi):
                    nc.tensor.matmul(pga[:, :], xb[:, kc * P:(kc + 1) * P],
                                     wqkv_sb[:, kc * QKVW:kc * QKVW + 512],
                                     start=(kc == 0), stop=(kc == KC - 1))
                    nc.tensor.matmul(pgb[:, :], xb[:, kc * P:(kc + 1) * P],
                                     wqkv_sb[:, kc * QKVW + 512:(kc + 1) * QKVW],
                                     start=(kc == 0), stop=(kc == KC - 1))
                    if ti_box[0] < len(tq) and kc in (3, 6, 9, 12, 15):
                        tq[ti_box[0]]()
                        ti_box[0] += 1
                if kc_hi == KC:
                    while ti_box[0] < len(tq):
                        tq[ti_box[0]]()
                        ti_box[0] += 1
                    # stream in a quarter of wo during phase A (needed from
                    # A2 on; too big to load in one go at that point)
                    if sb in (2, 4, 6, 8):
                        c = (sb - 2) // 2
                        nc.sync.dma_start(out=wo_sb[:, c * HID:(c + 1) * HID],
                                          in_=wo[:, c * HID:(c + 1) * HID])
                    rope_and_v(sb, pga, pgb)
                    del xb_tiles[(sb, "ps")]
                    del xb_tiles[sb]

            def proj(psg_p, sb, tq=()):
                proj_mms(psg_p, sb, 0, KC, tq, [0])

            def emit_scores(ps_p, t, qb, kb):
                # head pair (2t, 2t+1): concurrent K=64 matmuls on row
                # groups (0,0)/(64,0); outputs in the two banks of one
                # [128,1024] f32 tile; one exp covers both halves (incl.
                # the stale gap cols on diagonal rounds - never read back)
                kv = t // 2
                j0 = max(kb - 4 * qb, 0)
                sT = ps_p.tile([P, 1024], F32, tag="sT", name="sT")
                nc.tensor.matmul(sT[:, j0 * P:512],
                                 kT_sb[kv][0:D, kb * P:(kb + 1) * P],
                                 qT_sb[t][0:D, qb * 512 + j0 * P:(qb + 1) * 512],
                                 start=True, stop=True)
                nc.tensor.matmul(sT[:, 512 + j0 * P:1024],
                                 kT_sb[kv][D:P, kb * P:(kb + 1) * P],
                                 qT_sb[t][D:P, qb * 512 + j0 * P:(qb + 1) * 512],
                                 start=True, stop=True)
                ex = exs_p.tile([P, 1024], BF16, tag="ex2", name="ex2")
                nc.scalar.activation(ex[:, j0 * P:1024], sT[:, j0 * P:1024],
                                     AF.Exp, scale=0.125)
                return ex

            def emit_ctx(ctxA, ctxB, t, qb, kb, nkb, ex):
                kv = t // 2
                j0 = max(kb - 4 * qb, 0)
                if kb >= 4 * qb:
                    nc.vector.tensor_mul(ex[:, j0 * P:(j0 + 1) * P],
                                         ex[:, j0 * P:(j0 + 1) * P], tri01[:, :])
                    nc.vector.tensor_mul(ex[:, 512 + j0 * P:512 + (j0 + 1) * P],
                                         ex[:, 512 + j0 * P:512 + (j0 + 1) * P], tri01[:, :])
                va = vaug_sb[:, kb * 130 + kv * 65: kb * 130 + (kv + 1) * 65]
                nc.tensor.matmul(ctxA[:, j0 * P:512], va, ex[:, j0 * P:512],
                                 start=(kb == 0), stop=(kb == nkb - 1),
                                 skip_group_check=True)
                nc.tensor.matmul(ctxB[:, j0 * P:512], va, ex[:, 512 + j0 * P:1024],
                                 start=(kb == 0), stop=(kb == nkb - 1),
                                 skip_group_check=True)
                if kb == nkb - 1:
                    normalize(ctxA, t, 0, qb)
                    normalize(ctxB, t, D, qb)

            # ======== phase A: proj 0..13 + attention groups 0,1 ========
            with tc.tile_pool(name="psg", bufs=2, space="PSUM") as psg_p, \
                 tc.tile_pool(name="pst", bufs=1, space="PSUM") as pst_p, \
                 tc.tile_pool(name="pssA", bufs=2, space="PSUM") as pssA_p, \
                 tc.tile_pool(name="pscA", bufs=1, space="PSUM") as pscA_p:

                def attn_unit_a(h, qb):
                    t, roff, kv = h // 2, D * (h % 2), h // 4
                    qT = qT_sb[t][roff:roff + D, :]
                    kT = kT_sb[kv][roff:roff + D, :]
                    ctx = pscA_p.tile([65, 512], F32, tag="ctx", name="ctx")
                    nkb = 4 * qb + 4
                    for kb in range(nkb):
                        j0 = max(kb - 4 * qb, 0)
                        sT = pssA_p.tile([P, 512], F32, tag="sT", name="sT")
                        nc.tensor.matmul(sT[:, j0 * P:512], kT[:, kb * P:(kb + 1) * P],
                                         qT[:, qb * 512 + j0 * P:(qb + 1) * 512], start=True, stop=True)
                        ex = exs_p.tile([P, 512], BF16, tag="ex", name="ex")
                        nc.scalar.activation(ex[:, j0 * P:512], sT[:, j0 * P:512], AF.Exp, scale=0.125)
                        if kb >= 4 * qb:
                            nc.vector.tensor_mul(ex[:, j0 * P:(j0 + 1) * P],
                                                 ex[:, j0 * P:(j0 + 1) * P], tri01[:, :])
                        nc.tensor.matmul(ctx[:, j0 * P:512],
                                         vaug_sb[:, kb * 130 + kv * 65: kb * 130 + (kv + 1) * 65],
                                         ex[:, j0 * P:512], start=(kb == 0), stop=(kb == nkb - 1),
                                         skip_group_check=True)
                    normalize(ctx, t, roff, qb)

                fa = lambda sb: flip_closures(pst_p, "pt", sb)
                proj(psg_p, 0)
                proj(psg_p, 1)
                for sb in (2, 3, 4, 5):
                    proj(psg_p, sb, fa(sb - 2))
                # group 0 (qb=0) with proj 6..9, two units per proj
                proj(psg_p, 6, fa(4))
                attn_unit_a(0, 0)
                attn_unit_a(1, 0)
                proj(psg_p, 7, fa(5))
                attn_unit_a(2, 0)
                attn_unit_a(3, 0)
                proj(psg_p, 8, fa(6))
                attn_unit_a(4, 0)
                attn_unit_a(5, 0)
                proj(psg_p, 9, fa(7))
                attn_unit_a(6, 0)
                attn_unit_a(7, 0)
                # group 1 (qb=1) with proj 10..13, two units per proj
                proj(psg_p, 10, fa(8))
                attn_unit_a(0, 1)
                attn_unit_a(1, 1)
                proj(psg_p, 11, fa(9))
                attn_unit_a(2, 1)
                attn_unit_a(3, 1)
                proj(psg_p, 12, fa(10))
                attn_unit_a(4, 1)
                wo_unit(pst_p, 0, 0, tag="pt")
                attn_unit_a(5, 1)
                wo_unit(pst_p, 0, 1, tag="pt")
                proj(psg_p, 13, fa(11))
                attn_unit_a(6, 1)
                wo_unit(pst_p, 0, 2, tag="pt")
                wo_unit(pst_p, 1, 0, tag="pt")
                attn_unit_a(7, 1)
                # wo for sb 0,1 needs only qb group 0; it fills the PE while
                # the last qb1 units' exp drains on ACT
                wo_unit(pst_p, 0, 3, tag="pt")
                wo_unit(pst_p, 1, 1, tag="pt")
                wo_unit(pst_p, 1, 2, tag="pt")
                wo_unit(pst_p, 1, 3, tag="pt")

            # ======== phase A2: qb group 2 pair rounds + proj 14,15 ========
            # + flips 12..15 + wo for seq blocks 0..3
            with tc.tile_pool(name="psg2", bufs=1, space="PSUM") as psg2_p, \
                 tc.tile_pool(name="ps2a", bufs=1, space="PSUM") as ps2a_p, \
                 tc.tile_pool(name="psca", bufs=1, space="PSUM") as psca_p, \
                 tc.tile_pool(name="psoa", bufs=2, space="PSUM") as psoa_p:

                # filler items, emitted between attention rounds
                items = []
                ti14, ti15 = [0], [0]
                tq14 = flip_closures(psoa_p, "po", 12)
                tq15 = flip_closures(psoa_p, "po", 13)
                for piece in range(4):
                    items.append(lambda p=piece: proj_mms(psg2_p, 14, p * 4, (p + 1) * 4, tq14, ti14))
                for piece in range(4):
                    items.append(lambda p=piece: proj_mms(psg2_p, 15, p * 4, (p + 1) * 4, tq15, ti15))
                for f in flip_closures(psoa_p, "po", 14):
                    items.append(f)
                for f in flip_closures(psoa_p, "po", 15):
                    items.append(f)
                for sb in (2, 3):
                    for n in range(4):
                        items.append(lambda sb=sb, n=n: wo_unit(psoa_p, sb, n))

                rounds = [(t, 2, kb, 12) for t in range(4) for kb in range(12)]
                ctx_tiles = {}

                def get_ctx(pool, kb):
                    if kb == 0:
                        ctx_tiles[0] = pool.tile([65, 512], F32, tag="ctxA", name="ctxA")
                        ctx_tiles[1] = pool.tile([65, 512], F32, tag="ctxB", name="ctxB")
                    return ctx_tiles[0], ctx_tiles[1]

                ii = 0
                n_spread = len(items) - 8   # last 8 bridge the A2->B seam
                pend = None
                for ri, (t, qb, kb, nkb) in enumerate(rounds):
                    ex = emit_scores(ps2a_p, t, qb, kb)
                    while ii < n_spread and ii < (ri + 1) * n_spread // len(rounds):
                        items[ii]()
                        ii += 1
                    if pend is not None:
                        (pt_, pqb, pkb, pnkb, pex) = pend
                        ctxA, ctxB = get_ctx(psca_p, pkb)
                        emit_ctx(ctxA, ctxB, pt_, pqb, pkb, pnkb, pex)
                    pend = (t, qb, kb, nkb, ex)
                if pend is not None:
                    (pt_, pqb, pkb, pnkb, pex) = pend
                    ctxA, ctxB = get_ctx(psca_p, pkb)
                    emit_ctx(ctxA, ctxB, pt_, pqb, pkb, pnkb, pex)
                while ii < len(items):
                    items[ii]()
                    ii += 1

            # ======== phase B: qb group 3 pair rounds + wo 4..15 ========
            with tc.tile_pool(name="ps2", bufs=2, space="PSUM") as ps2_p, \
                 tc.tile_pool(name="psc", bufs=1, space="PSUM") as psc_p, \
                 tc.tile_pool(name="pso", bufs=2, space="PSUM") as pso_p:

                wos = [(sb, n) for sb in range(4, 16) for n in range(4)]
                wi = 0

                rounds = [(t, 3, kb, 16) for t in range(4) for kb in range(16)]
                ctx_tiles2 = {}

                def get_ctx2(kb):
                    if kb == 0:
                        ctx_tiles2[0] = psc_p.tile([65, 512], F32, tag="ctxA", name="ctxA")
                        ctx_tiles2[1] = psc_p.tile([65, 512], F32, tag="ctxB", name="ctxB")
                    return ctx_tiles2[0], ctx_tiles2[1]

                # software pipeline: scores r+1 before ctx r; wo for sb<12
                # (ready since A2) paced ~1 per 2 rounds, holding 2 back to
                # cover the rounds->tail boundary; sb>=12 gates on qb3 ctxT
                # and runs in the tail over rotated psum slots.
                pend = None
                for ri, (t, qb, kb, nkb) in enumerate(rounds):
                    ex = emit_scores(ps2_p, t, qb, kb)
                    if wi < 26 and ri % 2 == 0:
                        wo_unit(pso_p, *wos[wi])
                        wi += 1
                    if pend is not None:
                        (pt_, pqb, pkb, pnkb, pex) = pend
                        ctxA, ctxB = get_ctx2(pkb)
                        emit_ctx(ctxA, ctxB, pt_, pqb, pkb, pnkb, pex)
                    pend = (t, qb, kb, nkb, ex)
                if pend is not None:
                    (pt_, pqb, pkb, pnkb, pex) = pend
                    ctxA, ctxB = get_ctx2(pkb)
                    emit_ctx(ctxA, ctxB, pt_, pqb, pkb, pnkb, pex)
                # tail: the 2 held-back units bridge the normalize latency,
                # then sb 12..15 over rotated psum slots, copies alternating
                # between the idle scalar and vector engines
                tail_slots = [(pso_p, "po"), (psc_p, "ctxA"), (ps2_p, "sT"),
                              (pso_p, "po"), (psc_p, "ctxB"), (ps2_p, "sT")]
                for i, u in enumerate(wos[wi:]):
                    pool, tag = tail_slots[i % len(tail_slots)]
                    wo_unit(pool, *u, copy_eng=("s" if i % 2 else "v"), tag=tag)


def _build():
    nc = bacc.Bacc("TRN2", target_bir_lowering=False, debug=False, num_devices=8)
    # all inputs host-pre-permuted into DMA-friendly layouts (fat packets):
    # xT[sb*P+p, kc*P+j] = x.T[kc*P+p, sb*P+j]; weights/tables partition-major
    xT = nc.dram_tensor("xT", [S, HID], BF16, kind="ExternalInput").ap()
    wqkv = nc.dram_tensor("wqkv", [P, KC * QKVW], BF16, kind="ExternalInput").ap()
    wo = nc.dram_tensor("wo", [P, 4 * HID], BF16, kind="ExternalInput").ap()
    cosb = nc.dram_tensor("cosb", [P, SB * 32], BF16, kind="ExternalInput").ap()
    sinb = nc.dram_tensor("sinb", [P, SB * 32], BF16, kind="ExternalInput").ap()
    out = nc.dram_tensor("out", [S, HID], BF16, kind="ExternalOutput").ap()
    with tile.TileContext(nc) as tc:
        _emit_graph(nc, tc, xT, wqkv, wo, cosb, sinb, out)
    nc.finalize()
    return nc


def _planar(w, nheads):
    """Permute each head's 64 cols from interleaved (r0,i0,r1,i1,...) to
    planar (r0..r31, i0..i31)."""
    h = w.reshape(w.shape[0], nheads, 32, 2)
    return np.ascontiguousarray(h.transpose(0, 1, 3, 2).reshape(w.shape[0], nheads * 64))


def kernel(x, wq, wk, wv, wo, freqs_cos, freqs_sin, mask):
    x = np.asarray(x, dtype=np.float32)
    wq = np.asarray(wq, dtype=np.float32)
    wk = np.asarray(wk, dtype=np.float32)
    wv = np.asarray(wv, dtype=np.float32)
    wo = np.asarray(wo, dtype=np.float32)
    fc = np.asarray(freqs_cos, dtype=np.float32)
    fs = np.asarray(freqs_sin, dtype=np.float32)

    if "nc" not in _CACHE:
        _CACHE["nc"] = _build()
    nc = _CACHE["nc"]

    wqp = _planar(wq, 32)   # planar per q head
    wkp = _planar(wk, 8)    # planar per kv head
    cosb = np.ascontiguousarray(fc.reshape(SB, P, 32).transpose(1, 0, 2).reshape(P, SB * 32)).astype(BF)
    sinb = np.ascontiguousarray(fs.reshape(SB, P, 32).transpose(1, 0, 2).reshape(P, SB * 32)).astype(BF)
    in_maps = []
    for core in range(8):
        b, t = core // 4, core % 4
        # xT[sb*P+p, kc*P+j] = x[b][sb*P+j, kc*P+p]
        xTb = x[b].reshape(SB, P, KC, P).transpose(0, 3, 2, 1).reshape(S, HID)
        wqkv_full = np.concatenate(
            [wkp[:, t * KW:(t + 1) * KW],
             wqp[:, t * QW:(t + 1) * QW],
             wv[:, t * VW:(t + 1) * VW]], axis=1)   # [HID, 768]
        wqkv_p = wqkv_full.reshape(KC, P, QKVW).transpose(1, 0, 2).reshape(P, KC * QKVW)
        wo_p = wo[t * QW:(t + 1) * QW, :].reshape(4, P, HID).transpose(1, 0, 2).reshape(P, 4 * HID)
        in_maps.append({
            "xT": np.ascontiguousarray(xTb).astype(BF),
            "wqkv": np.ascontiguousarray(wqkv_p).astype(BF),
            "wo": np.ascontiguousarray(wo_p).astype(BF),
            "cosb": cosb, "sinb": sinb,
        })
    trace = bool(_CACHE.get("trace"))
    try:
        res = run_bass_kernel_spmd(nc, in_maps, list(range(8)), trace=trace)
    except Exception:
        if not trace:
            raise
        res = run_bass_kernel_spmd(nc, in_maps, list(range(8)))
    _CACHE["last_result"] = res
    outs = [np.asarray(r["out"], dtype=np.float32) for r in res.results]
    full = np.stack([outs[0] + outs[1] + outs[2] + outs[3],
                     outs[4] + outs[5] + outs[6] + outs[7]], axis=0)
    return full


# revision 11
# speedup vs baseline: 1.0194x; 1.0064x over previous
"""Distributed Bass kernel for llama-style GQA attention on 8 trn2 NeuronCores.

Sharding: 2-way data-parallel over batch x 4-way tensor-parallel over heads.
Core c handles batch b=c//4 and head group t=c%4 (8 q-heads, 2 kv-heads).
wq/wk/wv split column-wise per head group; wo split row-wise; each core
produces a partial [S, HIDDEN] bf16 output, host sums the 4 partials per batch.

On-chip flow per core (all matmuls bf16, psum f32):
  xT (pre-transposed on host) @ wqkv -> k|q|v per 128-seq block; RoPE in
  planar layout (host pre-permutes wq/wk cols to [real|imag] halves); q,k
  flipped to [d, seq] via regular matmuls against a moving identity; v kept
  [seq, d] with appended ones cols (free softmax denominators).
  Attention pair rounds: heads (2t, 2t+1) share a kv head and live in
  opposite 64-partition halves of qT/kT, so their K=64 score matmuls
  auto-derive tile_position (0,0)/(64,0) and execute CONCURRENTLY on
  disjoint PE row-groups; one exp activation covers the pair's [128,1024]
  score psum (stale gap cols on diagonal rounds are exp'd but never read).
  The kernel is ACT(exp)-bound during attention (~1.1us exp vs ~0.75us PE
  per round), so proj/flip/wo matmul filler is spread across ALL attention:
    phase A : proj 0..13 + qb groups 0,1 (single-head units, baseline psum)
    phase A2: qb group 2 pair rounds + proj 14,15 (split in 4 pieces) +
              flips 12..15 + wo for seq blocks 0..3
    phase B : qb group 3 pair rounds (sT double-buffered, scores r+1
              emitted before ctx r) + wo for seq blocks 4..15 paced evenly
"""

import numpy as np
import ml_dtypes

import concourse.bass as bass
import concourse.mybir as mybir
import concourse.tile as tile
from concourse import bacc
from concourse.bass_utils import run_bass_kernel_spmd
from concourse.masks import make_identity

B, S, HID = 2, 2048, 2048
D = 64
NQ, NKV = 8, 2          # per-core heads
KW, QW, VW = NKV * D, NQ * D, NKV * D
QKVW = KW + QW + VW     # 768, layout [k(128) | q(512) | v(128)]
P = 128
SB = S // P             # 16 seq blocks
KC = HID // P           # 16 contraction chunks
F32 = mybir.dt.float32
BF16 = mybir.dt.bfloat16
BF = ml_dtypes.bfloat16
AF = mybir.ActivationFunctionType

_CACHE = {}


def _emit_graph(nc, tc, xT, wqkv, wo, cosb, sinb, out):
    with tc.tile_pool(name="const", bufs=1) as const, \
         tc.tile_pool(name="big", bufs=1) as big:
        # persistent tensors
        qT_sb = [big.tile([P, S], BF16, tag=f"qT{t}", name=f"qT{t}") for t in range(4)]
        kT_sb = [big.tile([P, S], BF16, tag=f"kT{k}", name=f"kT{k}") for k in range(NKV)]
        # vaug layout: per (sb, kv): 65 cols (64 v dims + 1 ones col)
        vaug_sb = big.tile([P, SB * NKV * 65], BF16, tag="va")
        ctxT_sb = [big.tile([P, S], BF16, tag=f"cT{t}", name=f"cT{t}") for t in range(4)]
        rot2 = big.tile([P, SB * 640], BF16, tag="rot2")   # roped k|q per sb
        wqkv_sb = big.tile([P, KC * QKVW], BF16, tag="wqkv")
        wo_sb = big.tile([P, 4 * HID], BF16, tag="wo")
        cos_sb = big.tile([P, SB * 32], BF16, tag="c1")
        sin_sb = big.tile([P, SB * 32], BF16, tag="s1")

        ident = const.tile([P, P], BF16, tag="id")
        make_identity(nc, ident[:, :])
        # tri01[k, q] = 1 where q >= k else 0 (keep-mask for aligned diag blocks)
        tri01 = const.tile([P, P], BF16, tag="tri")
        nc.gpsimd.memset(tri01[:, :], 1.0)
        nc.gpsimd.affine_select(
            out=tri01[:, :], in_=tri01[:, :], compare_op=mybir.AluOpType.is_ge,
            fill=0.0, base=0, pattern=[[1, P]], channel_multiplier=-1,
        )
        ones_cols = vaug_sb[:, :].rearrange("p (b c) -> p b c", c=65)[:, :, 64:65]
        nc.gpsimd.memset(ones_cols, 1.0)

        with tc.tile_pool(name="xp", bufs=2) as xp_p, \
             tc.tile_pool(name="rt", bufs=2) as rt_p, \
             tc.tile_pool(name="exs", bufs=6) as exs_p, \
             tc.tile_pool(name="nrm", bufs=6) as nrm_p, \
             tc.tile_pool(name="osb", bufs=6) as osb_p:

            xb_tiles = {}

            def xb_load(sb):
                xb = xp_p.tile([P, KC * P], BF16, tag="xb", name="xb")
                nc.sync.dma_start(out=xb[:, :], in_=xT[sb * P:(sb + 1) * P, :])
                xb_tiles[sb] = xb

            # first proj matmul gates only on xb chunk 0 + wqkv kc-block 0
            # (subtile deps), so load those first and stream the rest behind
            xb0 = xp_p.tile([P, KC * P], BF16, tag="xb", name="xb")
            nc.sync.dma_start(out=xb0[:, 0:512], in_=xT[0:P, 0:512])
            nc.sync.dma_start(out=wqkv_sb[:, 0:QKVW], in_=wqkv[:, 0:QKVW])
            nc.sync.dma_start(out=wqkv_sb[:, QKVW:4 * QKVW],
                              in_=wqkv[:, QKVW:4 * QKVW])
            nc.sync.dma_start(out=xb0[:, 512:2048], in_=xT[0:P, 512:2048])
            for c in range(1, 4):
                nc.sync.dma_start(out=wqkv_sb[:, c * 4 * QKVW:(c + 1) * 4 * QKVW],
                                  in_=wqkv[:, c * 4 * QKVW:(c + 1) * 4 * QKVW])
            xb_tiles[0] = xb0
            nc.sync.dma_start(out=cos_sb[:, :], in_=cosb)
            nc.sync.dma_start(out=sin_sb[:, :], in_=sinb)

            def rope_and_v(sb, pga, pgb):
                rsrc = rt_p.tile([P, KW + QW], BF16, tag="rsrc", name="rsrc")
                nc.vector.tensor_copy(rsrc[:, 0:512], pga[:, :])
                nc.vector.tensor_copy(rsrc[:, 512:KW + QW], pgb[:, 0:128])
                nh = NKV + NQ
                rs = rsrc[:, :].rearrange("p (h i) -> p h i", h=nh)
                ev, od = rs[:, :, 0:32], rs[:, :, 32:64]
                c = cos_sb[:, sb * 32:(sb + 1) * 32].rearrange("p (o i) -> p o i", o=1).broadcast_to((P, nh, 32))
                s = sin_sb[:, sb * 32:(sb + 1) * 32].rearrange("p (o i) -> p o i", o=1).broadcast_to((P, nh, 32))
                t1 = rt_p.tile([P, nh * 32], BF16, tag="t1", name="t1")
                t2 = rt_p.tile([P, nh * 32], BF16, tag="t2", name="t2")
                t1r = t1[:, :].rearrange("p (h i) -> p h i", h=nh)
                t2r = t2[:, :].rearrange("p (h i) -> p h i", h=nh)
                dst = rot2[:, sb * 640:(sb + 1) * 640].rearrange("p (h i) -> p h i", h=nh)
                dst_e, dst_o = dst[:, :, 0:32], dst[:, :, 32:64]
                nc.vector.tensor_mul(t1r, ev, c)
                nc.vector.tensor_mul(t2r, od, s)
                nc.vector.tensor_sub(dst_e, t1r, t2r)
                nc.vector.tensor_mul(t1r, ev, s)
                nc.vector.tensor_mul(t2r, od, c)
                nc.vector.tensor_add(dst_o, t1r, t2r)
                vdst = vaug_sb[:, sb * 130: sb * 130 + 130].rearrange("p (kv c) -> p kv c", kv=2)[:, :, 0:D]
                vsrc = pgb[:, 128:256].rearrange("p (kv c) -> p kv c", kv=2)
                nc.vector.tensor_copy(vdst, vsrc)

            def wo_unit(pso_p, sb, n, copy_eng="v", tag="po"):
                po = pso_p.tile([P, 512], F32, tag=tag, name="po")
                for c in range(4):
                    nc.tensor.matmul(po[:, :], ctxT_sb[c][:, sb * P:(sb + 1) * P],
                                     wo_sb[:, c * HID + n * 512: c * HID + (n + 1) * 512],
                                     start=(c == 0), stop=(c == 3))
                ob = osb_p.tile([P, 512], BF16, tag="ob", name="ob")
                if copy_eng == "s":
                    nc.scalar.activation(ob[:, :], po[:, :], AF.Copy)
                else:
                    nc.vector.tensor_copy(ob[:, :], po[:, :])
                eng = nc.gpsimd if (sb + n) % 2 else nc.sync
                eng.dma_start(out=out[sb * P:(sb + 1) * P, n * 512:(n + 1) * 512],
                              in_=ob[:, :])

            def normalize(ctx, t, roff, qb):
                # copy ctx out of psum immediately (den + staged numerator)
                # so the pair's ctx psum frees after ~1us instead of after
                # the whole broadcast/recip/mul chain - the next unit's
                # first ctx matmul WAR-waits on this release
                den = nrm_p.tile([1, 512], F32, tag="den", name="den")
                nc.vector.tensor_copy(den[:, :], ctx[64:65, :])
                cstg = nrm_p.tile([D, 512], BF16, tag="cstg", name="cstg")
                nc.vector.tensor_copy(cstg[:, :], ctx[0:D, :])
                dbc = nrm_p.tile([D, 512], F32, tag="dbc", name="dbc")
                nc.gpsimd.partition_broadcast(dbc[:, :], den[:, :])
                rbc = nrm_p.tile([D, 512], F32, tag="rbc", name="rbc")
                nc.vector.reciprocal_approx_fast(out=rbc[:, :], in_=dbc[:, :])
                ntmp = nrm_p.tile([D, 512], BF16, tag="ntmp", name="ntmp")
                # qb=3 in two chunks: the tail wo units for sb 12,13 gate on
                # the first half of every head's last ctxT block
                cw = (256,) if qb == 3 else (512,)
                c0 = 0
                for w in (cw * 2 if qb == 3 else cw):
                    nc.vector.tensor_mul(ntmp[:, c0:c0 + w], cstg[:, c0:c0 + w], rbc[:, c0:c0 + w])
                    nc.sync.dma_start(
                        out=ctxT_sb[t][roff:roff + D, qb * 512 + c0:qb * 512 + c0 + w],
                        in_=ntmp[:, c0:c0 + w])
                    c0 += w

            def flip_closures(pt_pool, pt_tag, sb):
                # q/k flips: regular matmul with the block as the stationary
                # operand and identity as the moving one (out = blk.T @ I).
                def t_k():
                    pt = pt_pool.tile([P, P], F32, tag=pt_tag, name="pt")
                    nc.tensor.matmul(pt[:, :], rot2[:, sb * 640: sb * 640 + P],
                                     ident[:, :], start=True, stop=True)
                    nc.vector.tensor_copy(kT_sb[0][0:D, sb * P:(sb + 1) * P], pt[0:D, :])
                    nc.vector.tensor_copy(kT_sb[1][D:P, sb * P:(sb + 1) * P], pt[D:P, :])
                    nc.sync.dma_start(out=kT_sb[0][D:P, sb * P:(sb + 1) * P],
                                      in_=kT_sb[0][0:D, sb * P:(sb + 1) * P])
                    nc.sync.dma_start(out=kT_sb[1][0:D, sb * P:(sb + 1) * P],
                                      in_=kT_sb[1][D:P, sb * P:(sb + 1) * P])

                def t_q(t):
                    def f():
                        pt = pt_pool.tile([P, P], F32, tag=pt_tag, name="pt")
                        nc.tensor.matmul(pt[:, :],
                                         rot2[:, sb * 640 + KW + t * P: sb * 640 + KW + (t + 1) * P],
                                         ident[:, :], start=True, stop=True)
                        nc.vector.tensor_copy(qT_sb[t][:, sb * P:(sb + 1) * P], pt[:, :])
                    return f
                return [t_k] + [t_q(t) for t in range(4)]

            def proj_mms(psg_p, sb, kc_lo, kc_hi, tq, ti_box):
                """One piece of the qkv projection for seq block sb."""
                if kc_lo == 0:
                    if sb not in xb_tiles:
                        xb_load(sb)
                    if sb + 1 <= 15 and sb + 1 not in xb_tiles:
                        xb_load(sb + 1)   # prefetch next block
                    pga = psg_p.tile([P, 512], F32, tag="pga", name="pga")
                    pgb = psg_p.tile([P, 256], F32, tag="pgb", name="pgb")
                    xb_tiles[(sb, "ps")] = (pga, pgb)
                pga, pgb = xb_tiles[(sb, "ps")]
                xb = xb_tiles[sb]
                for kc in range(kc_lo, kc_hi):
                    nc.tensor.matmul(pga[:, :], xb[:, kc * P:(kc + 1) * P],
                                     wqkv_sb[:, kc * QKVW:kc * QKVW + 512],
                                     start=(kc == 0), stop=(kc == KC - 1))
                    nc.tensor.matmul(pgb[:, :], xb[:, kc * P:(kc + 1) * P],
                                     wqkv_sb[:, kc * QKVW + 512:(kc + 1) * QKVW],
                                     start=(kc == 0), stop=(kc == KC - 1))
                    if ti_box[0] < len(tq) and kc in (3, 6, 9, 12, 15):
                        tq[ti_box[0]]()
                        ti_box[0] += 1
                if kc_hi == KC:
                    while ti_box[0] < len(tq):
                        tq[ti_box[0]]()
                        ti_box[0] += 1
                    # stream in a quarter of wo during phase A (needed from
                    # A2 on; too big to load in one go at that point)
                    if sb in (2, 4, 6, 8):
                        c = (sb - 2) // 2
                        nc.sync.dma_start(out=wo_sb[:, c * HID:(c + 1) * HID],
                                          in_=wo[:, c * HID:(c + 1) * HID])
                    rope_and_v(sb, pga, pgb)
                    del xb_tiles[(sb, "ps")]
                    del xb_tiles[sb]

            def proj(psg_p, sb, tq=()):
                proj_mms(psg_p, sb, 0, KC, tq, [0])

            def emit_scores(ps_p, t, qb, kb):
                # head pair (2t, 2t+1): concurrent K=64 matmuls on row
                # groups (0,0)/(64,0); outputs in the two banks of one
                # [128,1024] f32 tile; one exp covers both halves (incl.
                # the stale gap cols on diagonal rounds - never read back)
                kv = t // 2
                j0 = max(kb - 4 * qb, 0)
                sT = ps_p.tile([P, 1024], F32, tag="sT", name="sT")
                nc.tensor.matmul(sT[:, j0 * P:512],
                                 kT_sb[kv][0:D, kb * P:(kb + 1) * P],
                                 qT_sb[t][0:D, qb * 512 + j0 * P:(qb + 1) * 512],
                                 start=True, stop=True)
                nc.tensor.matmul(sT[:, 512 + j0 * P:1024],
                                 kT_sb[kv][D:P, kb * P:(kb + 1) * P],
                                 qT_sb[t][D:P, qb * 512 + j0 * P:(qb + 1) * 512],
                                 start=True, stop=True)
                ex = exs_p.tile([P, 1024], BF16, tag="ex2", name="ex2")
                nc.scalar.activation(ex[:, j0 * P:1024], sT[:, j0 * P:1024],
                                     AF.Exp, scale=0.125)
                return ex

            def emit_ctx(ctxA, ctxB, t, qb, kb, nkb, ex):
                kv = t // 2
                j0 = max(kb - 4 * qb, 0)
                if kb >= 4 * qb:
                    nc.vector.tensor_mul(ex[:, j0 * P:(j0 + 1) * P],
                                         ex[:, j0 * P:(j0 + 1) * P], tri01[:, :])
                    nc.vector.tensor_mul(ex[:, 512 + j0 * P:512 + (j0 + 1) * P],
                                         ex[:, 512 + j0 * P:512 + (j0 + 1) * P], tri01[:, :])
                va = vaug_sb[:, kb * 130 + kv * 65: kb * 130 + (kv + 1) * 65]
                nc.tensor.matmul(ctxA[:, j0 * P:512], va, ex[:, j0 * P:512],
                                 start=(kb == 0), stop=(kb == nkb - 1),
                                 skip_group_check=True)
                nc.tensor.matmul(ctxB[:, j0 * P:512], va, ex[:, 512 + j0 * P:1024],
                                 start=(kb == 0), stop=(kb == nkb - 1),
                                 skip_group_check=True)
                if kb == nkb - 1:
                    normalize(ctxA, t, 0, qb)
                    normalize(ctxB, t, D, qb)

            # ======== phase A: proj 0..13 + attention groups 0,1 ========
            with tc.tile_pool(name="psg", bufs=2, space="PSUM") as psg_p, \
                 tc.tile_pool(name="pst", bufs=1, space="PSUM") as pst_p, \
                 tc.tile_pool(name="pssA", bufs=2, space="PSUM") as pssA_p, \
                 tc.tile_pool(name="pscA", bufs=1, space="PSUM") as pscA_p:

                def attn_unit_a(h, qb):
                    t, roff, kv = h // 2, D * (h % 2), h // 4
                    qT = qT_sb[t][roff:roff + D, :]
                    kT = kT_sb[kv][roff:roff + D, :]
                    ctx = pscA_p.tile([65, 512], F32, tag="ctx", name="ctx")
                    nkb = 4 * qb + 4
                    for kb in range(nkb):
                        j0 = max(kb - 4 * qb, 0)
                        sT = pssA_p.tile([P, 512], F32, tag="sT", name="sT")
                        nc.tensor.matmul(sT[:, j0 * P:512], kT[:, kb * P:(kb + 1) * P],
                                         qT[:, qb * 512 + j0 * P:(qb + 1) * 512], start=True, stop=True)
                        ex = exs_p.tile([P, 512], BF16, tag="ex", name="ex")
                        nc.scalar.activation(ex[:, j0 * P:512], sT[:, j0 * P:512], AF.Exp, scale=0.125)
                        if kb >= 4 * qb:
                            nc.vector.tensor_mul(ex[:, j0 * P:(j0 + 1) * P],
                                                 ex[:, j0 * P:(j0 + 1) * P], tri01[:, :])
                        nc.tensor.matmul(ctx[:, j0 * P:512],
                                         vaug_sb[:, kb * 130 + kv * 65: kb * 130 + (kv + 1) * 65],
                                         ex[:, j0 * P:512], start=(kb == 0), stop=(kb == nkb - 1),
                                         skip_group_check=True)
                    normalize(ctx, t, roff, qb)

                fa = lambda sb: flip_closures(pst_p, "pt", sb)
                proj(psg_p, 0)
                proj(psg_p, 1)
                for sb in (2, 3, 4, 5):
                    proj(psg_p, sb, fa(sb - 2))
                # group 0 (qb=0) with proj 6..9, two units per proj
                proj(psg_p, 6, fa(4))
                attn_unit_a(0, 0)
                attn_unit_a(1, 0)
                proj(psg_p, 7, fa(5))
                attn_unit_a(2, 0)
                attn_unit_a(3, 0)
                proj(psg_p, 8, fa(6))
                attn_unit_a(4, 0)
                attn_unit_a(5, 0)
                proj(psg_p, 9, fa(7))
                attn_unit_a(6, 0)
                attn_unit_a(7, 0)
                # group 1 (qb=1) with proj 10..13, two units per proj
                proj(psg_p, 10, fa(8))
                attn_unit_a(0, 1)
                attn_unit_a(1, 1)
                proj(psg_p, 11, fa(9))
                attn_unit_a(2, 1)
                attn_unit_a(3, 1)
                proj(psg_p, 12, fa(10))
                attn_unit_a(4, 1)
                wo_unit(pst_p, 0, 0, tag="pt")
                attn_unit_a(5, 1)
                wo_unit(pst_p, 0, 1, tag="pt")
                proj(psg_p, 13, fa(11))
                attn_unit_a(6, 1)
                wo_unit(pst_p, 0, 2, tag="pt")
                wo_unit(pst_p, 1, 0, tag="pt")
                attn_unit_a(7, 1)
                # wo for sb 0,1 needs only qb group 0; it fills the PE while
                # the last qb1 units' exp drains on ACT
                wo_unit(pst_p, 0, 3, tag="pt")
                wo_unit(pst_p, 1, 1, tag="pt")

            # ======== phase A2: qb group 2 pair rounds + proj 14,15 ========
            # + flips 12..15 + wo for seq blocks 0..3
            with tc.tile_pool(name="psg2", bufs=1, space="PSUM") as psg2_p, \
                 tc.tile_pool(name="ps2a", bufs=1, space="PSUM") as ps2a_p, \
                 tc.tile_pool(name="psca", bufs=1, space="PSUM") as psca_p, \
                 tc.tile_pool(name="psoa", bufs=2, space="PSUM") as psoa_p:

                # filler items, emitted between attention rounds
                items = []
                items.append(lambda: wo_unit(psoa_p, 1, 2))
                items.append(lambda: wo_unit(psoa_p, 1, 3))
                ti14, ti15 = [0], [0]
                tq14 = flip_closures(psoa_p, "po", 12)
                tq15 = flip_closures(psoa_p, "po", 13)
                for piece in range(4):
                    items.append(lambda p=piece: proj_mms(psg2_p, 14, p * 4, (p + 1) * 4, tq14, ti14))
                for piece in range(4):
                    items.append(lambda p=piece: proj_mms(psg2_p, 15, p * 4, (p + 1) * 4, tq15, ti15))
                for f in flip_closures(psoa_p, "po", 14):
                    items.append(f)
                for f in flip_closures(psoa_p, "po", 15):
                    items.append(f)
                for sb in (2, 3):
                    for n in range(4):
                        items.append(lambda sb=sb, n=n: wo_unit(psoa_p, sb, n))

                rounds = [(t, 2, kb, 12) for t in range(4) for kb in range(12)]
                ctx_tiles = {}

                def get_ctx(pool, kb):
                    if kb == 0:
                        ctx_tiles[0] = pool.tile([65, 512], F32, tag="ctxA", name="ctxA")
                        ctx_tiles[1] = pool.tile([65, 512], F32, tag="ctxB", name="ctxB")
                    return ctx_tiles[0], ctx_tiles[1]

                n_spread = len(items) - 8   # last 8 bridge the A2->B seam
                items[0]()   # 2 wo units keep the PE dense across the
                items[1]()   # phase A -> A2 pool transition
                ii = 2
                pend = None
                for ri, (t, qb, kb, nkb) in enumerate(rounds):
                    ex = emit_scores(ps2a_p, t, qb, kb)
                    while ii < n_spread and ii < 2 + (ri + 1) * (n_spread - 2) // len(rounds):
                        items[ii]()
                        ii += 1
                    if pend is not None:
                        (pt_, pqb, pkb, pnkb, pex) = pend
                        ctxA, ctxB = get_ctx(psca_p, pkb)
                        emit_ctx(ctxA, ctxB, pt_, pqb, pkb, pnkb, pex)
                    pend = (t, qb, kb, nkb, ex)
                if pend is not None:
                    (pt_, pqb, pkb, pnkb, pex) = pend
                    ctxA, ctxB = get_ctx(psca_p, pkb)
                    emit_ctx(ctxA, ctxB, pt_, pqb, pkb, pnkb, pex)
                while ii < len(items):
                    items[ii]()
                    ii += 1

            # ======== phase B: qb group 3 pair rounds + wo 4..15 ========
            with tc.tile_pool(name="ps2", bufs=2, space="PSUM") as ps2_p, \
                 tc.tile_pool(name="psc", bufs=1, space="PSUM") as psc_p, \
                 tc.tile_pool(name="pso", bufs=2, space="PSUM") as pso_p:

                wos = [(sb, n) for sb in range(4, 16) for n in range(4)]
                wi = 0

                rounds = [(t, 3, kb, 16) for t in range(4) for kb in range(16)]
                ctx_tiles2 = {}

                def get_ctx2(kb):
                    if kb == 0:
                        ctx_tiles2[0] = psc_p.tile([65, 512], F32, tag="ctxA", name="ctxA")
                        ctx_tiles2[1] = psc_p.tile([65, 512], F32, tag="ctxB", name="ctxB")
                    return ctx_tiles2[0], ctx_tiles2[1]

                # software pipeline over round PAIRS: both rounds' score
                # pairs are emitted back-to-back (same row-group sequences
                # overlap their drains), then wo filler, then the previous
                # pair's ctx matmuls. wo cadence covers unit boundaries.
                pend = []
                it = 0
                for r0 in range(0, len(rounds), 2):
                    chunk = rounds[r0:r0 + 2]
                    exs = [(t, qb, kb, nkb, emit_scores(ps2_p, t, qb, kb))
                           for (t, qb, kb, nkb) in chunk]
                    if wi < 26 and it % 5 < 4:
                        wo_unit(pso_p, *wos[wi])
                        wi += 1
                    it += 1
                    for (pt_, pqb, pkb, pnkb, pex) in pend:
                        ctxA, ctxB = get_ctx2(pkb)
                        emit_ctx(ctxA, ctxB, pt_, pqb, pkb, pnkb, pex)
                    pend = exs
                for (pt_, pqb, pkb, pnkb, pex) in pend:
                    ctxA, ctxB = get_ctx2(pkb)
                    emit_ctx(ctxA, ctxB, pt_, pqb, pkb, pnkb, pex)
                # tail: the 2 held-back units bridge the normalize latency,
                # then sb 12..15 over rotated psum slots, copies alternating
                # between the idle scalar and vector engines
                tail_slots = [(pso_p, "po"), (psc_p, "ctxA"), (ps2_p, "sT"),
                              (pso_p, "po"), (psc_p, "ctxB"), (ps2_p, "sT")]
                for i, u in enumerate(wos[wi:]):
                    pool, tag = tail_slots[i % len(tail_slots)]
                    wo_unit(pool, *u, copy_eng=("s" if i % 2 else "v"), tag=tag)


def _build():
    nc = bacc.Bacc("TRN2", target_bir_lowering=False, debug=False, num_devices=8)
    # all inputs host-pre-permuted into DMA-friendly layouts (fat packets):
    # xT[sb*P+p, kc*P+j] = x.T[kc*P+p, sb*P+j]; weights/tables partition-major
    xT = nc.dram_tensor("xT", [S, HID], BF16, kind="ExternalInput").ap()
    wqkv = nc.dram_tensor("wqkv", [P, KC * QKVW], BF16, kind="ExternalInput").ap()
    wo = nc.dram_tensor("wo", [P, 4 * HID], BF16, kind="ExternalInput").ap()
    cosb = nc.dram_tensor("cosb", [P, SB * 32], BF16, kind="ExternalInput").ap()
    sinb = nc.dram_tensor("sinb", [P, SB * 32], BF16, kind="ExternalInput").ap()
    out = nc.dram_tensor("out", [S, HID], BF16, kind="ExternalOutput").ap()
    with tile.TileContext(nc) as tc:
        _emit_graph(nc, tc, xT, wqkv, wo, cosb, sinb, out)
    nc.finalize()
    return nc


def _planar(w, nheads):
    """Permute each head's 64 cols from interleaved (r0,i0,r1,i1,...) to
    planar (r0..r31, i0..i31)."""
    h = w.reshape(w.shape[0], nheads, 32, 2)
    return np.ascontiguousarray(h.transpose(0, 1, 3, 2).reshape(w.shape[0], nheads * 64))


def kernel(x, wq, wk, wv, wo, freqs_cos, freqs_sin, mask):
    x = np.asarray(x, dtype=np.float32)
    wq = np.asarray(wq, dtype=np.float32)
    wk = np.asarray(wk, dtype=np.float32)
    wv = np.asarray(wv, dtype=np.float32)
    wo = np.asarray(wo, dtype=np.float32)
    fc = np.asarray(freqs_cos, dtype=np.float32)
    fs = np.asarray(freqs_sin, dtype=np.float32)

    if "nc" not in _CACHE:
        _CACHE["nc"] = _build()
    nc = _CACHE["nc"]

    wqp = _planar(wq, 32)   # planar per q head
    wkp = _planar(wk, 8)    # planar per kv head
    cosb = np.ascontiguousarray(fc.reshape(SB, P, 32).transpose(1, 0, 2).reshape(P, SB * 32)).astype(BF)
    sinb = np.ascontiguousarray(fs.reshape(SB, P, 32).transpose(1, 0, 2).reshape(P, SB * 32)).astype(BF)
    in_maps = []
    for core in range(8):
        b, t = core // 4, core % 4
        # xT[sb*P+p, kc*P+j] = x[b][sb*P+j, kc*P+p]
        xTb = x[b].reshape(SB, P, KC, P).transpose(0, 3, 2, 1).reshape(S, HID)
        wqkv_full = np.concatenate(
            [wkp[:, t * KW:(t + 1) * KW],
             wqp[:, t * QW:(t + 1) * QW],
             wv[:, t * VW:(t + 1) * VW]], axis=1)   # [HID, 768]
        wqkv_p = wqkv_full.reshape(KC, P, QKVW).transpose(1, 0, 2).reshape(P, KC * QKVW)
        wo_p = wo[t * QW:(t + 1) * QW, :].reshape(4, P, HID).transpose(1, 0, 2).reshape(P, 4 * HID)
        in_maps.append({
            "xT": np.ascontiguousarray(xTb).astype(BF),
            "wqkv": np.ascontiguousarray(wqkv_p).astype(BF),
            "wo": np.ascontiguousarray(wo_p).astype(BF),
            "cosb": cosb, "sinb": sinb,
        })
    trace = bool(_CACHE.get("trace"))
    try:
        res = run_bass_kernel_spmd(nc, in_maps, list(range(8)), trace=trace)
    except Exception:
        if not trace:
            raise
        res = run_bass_kernel_spmd(nc, in_maps, list(range(8)))
    _CACHE["last_result"] = res
    outs = [np.asarray(r["out"], dtype=np.float32) for r in res.results]
    full = np.stack([outs[0] + outs[1] + outs[2] + outs[3],
                     outs[4] + outs[5] + outs[6] + outs[7]], axis=0)
    return full


# revision 12
# speedup vs baseline: 1.0227x; 1.0033x over previous
"""Distributed Bass kernel for llama-style GQA attention on 8 trn2 NeuronCores.

Sharding: 2-way data-parallel over batch x 4-way tensor-parallel over heads.
Core c handles batch b=c//4 and head group t=c%4 (8 q-heads, 2 kv-heads).
wq/wk/wv split column-wise per head group; wo split row-wise; each core
produces a partial [S, HIDDEN] bf16 output, host sums the 4 partials per batch.

On-chip flow per core (all matmuls bf16, psum f32):
  xT (pre-transposed on host) @ wqkv -> k|q|v per 128-seq block; RoPE in
  planar layout (host pre-permutes wq/wk cols to [real|imag] halves); q,k
  flipped to [d, seq] via regular matmuls against a moving identity; v kept
  [seq, d] with appended ones cols (free softmax denominators).
  Attention pair rounds: heads (2t, 2t+1) share a kv head and live in
  opposite 64-partition halves of qT/kT, so their K=64 score matmuls
  auto-derive tile_position (0,0)/(64,0) and execute CONCURRENTLY on
  disjoint PE row-groups; one exp activation covers the pair's [128,1024]
  score psum (stale gap cols on diagonal rounds are exp'd but never read).
  The kernel is ACT(exp)-bound during attention (~1.1us exp vs ~0.75us PE
  per round), so proj/flip/wo matmul filler is spread across ALL attention:
    phase A : proj 0..13 + qb groups 0,1 (single-head units, baseline psum)
    phase A2: qb group 2 pair rounds + proj 14,15 (split in 4 pieces) +
              flips 12..15 + wo for seq blocks 0..3
    phase B : qb group 3 pair rounds (sT double-buffered, scores r+1
              emitted before ctx r) + wo for seq blocks 4..15 paced evenly
"""

import numpy as np
import ml_dtypes

import concourse.bass as bass
import concourse.mybir as mybir
import concourse.tile as tile
from concourse import bacc
from concourse.bass_utils import run_bass_kernel_spmd
from concourse.masks import make_identity

B, S, HID = 2, 2048, 2048
D = 64
NQ, NKV = 8, 2          # per-core heads
KW, QW, VW = NKV * D, NQ * D, NKV * D
QKVW = KW + QW + VW     # 768, layout [k(128) | q(512) | v(128)]
P = 128
SB = S // P             # 16 seq blocks
KC = HID // P           # 16 contraction chunks
F32 = mybir.dt.float32
BF16 = mybir.dt.bfloat16
BF = ml_dtypes.bfloat16
AF = mybir.ActivationFunctionType

_CACHE = {}


def _emit_graph(nc, tc, xT, wqkv, wo, cosb, sinb, out):
    with tc.tile_pool(name="const", bufs=1) as const, \
         tc.tile_pool(name="big", bufs=1) as big:
        # persistent tensors
        qT_sb = [big.tile([P, S], BF16, tag=f"qT{t}", name=f"qT{t}") for t in range(4)]
        kT_sb = [big.tile([P, S], BF16, tag=f"kT{k}", name=f"kT{k}") for k in range(NKV)]
        # vaug layout: per (sb, kv): 65 cols (64 v dims + 1 ones col)
        vaug_sb = big.tile([P, SB * NKV * 65], BF16, tag="va")
        ctxT_sb = [big.tile([P, S], BF16, tag=f"cT{t}", name=f"cT{t}") for t in range(4)]
        rot2 = big.tile([P, SB * 640], BF16, tag="rot2")   # roped k|q per sb
        wqkv_sb = big.tile([P, KC * QKVW], BF16, tag="wqkv")
        wo_sb = big.tile([P, 4 * HID], BF16, tag="wo")
        cos_sb = big.tile([P, SB * 32], BF16, tag="c1")
        sin_sb = big.tile([P, SB * 32], BF16, tag="s1")

        ident = const.tile([P, P], BF16, tag="id")
        make_identity(nc, ident[:, :])
        # tri01[k, q] = 1 where q >= k else 0 (keep-mask for aligned diag blocks)
        tri01 = const.tile([P, P], BF16, tag="tri")
        nc.gpsimd.memset(tri01[:, :], 1.0)
        nc.gpsimd.affine_select(
            out=tri01[:, :], in_=tri01[:, :], compare_op=mybir.AluOpType.is_ge,
            fill=0.0, base=0, pattern=[[1, P]], channel_multiplier=-1,
        )
        ones_cols = vaug_sb[:, :].rearrange("p (b c) -> p b c", c=65)[:, :, 64:65]
        nc.gpsimd.memset(ones_cols, 1.0)

        with tc.tile_pool(name="xp", bufs=2) as xp_p, \
             tc.tile_pool(name="rt", bufs=2) as rt_p, \
             tc.tile_pool(name="exs", bufs=6) as exs_p, \
             tc.tile_pool(name="nrm", bufs=6) as nrm_p, \
             tc.tile_pool(name="osb", bufs=6) as osb_p:

            xb_tiles = {}

            def xb_load(sb):
                xb = xp_p.tile([P, KC * P], BF16, tag="xb", name="xb")
                nc.sync.dma_start(out=xb[:, :], in_=xT[sb * P:(sb + 1) * P, :])
                xb_tiles[sb] = xb

            # first proj matmul gates only on xb chunk 0 + wqkv kc-block 0
            # (subtile deps), so load those first and stream the rest behind
            xb0 = xp_p.tile([P, KC * P], BF16, tag="xb", name="xb")
            nc.sync.dma_start(out=xb0[:, 0:512], in_=xT[0:P, 0:512])
            nc.sync.dma_start(out=wqkv_sb[:, 0:QKVW], in_=wqkv[:, 0:QKVW])
            nc.sync.dma_start(out=wqkv_sb[:, QKVW:4 * QKVW],
                              in_=wqkv[:, QKVW:4 * QKVW])
            nc.sync.dma_start(out=xb0[:, 512:2048], in_=xT[0:P, 512:2048])
            for c in range(1, 4):
                nc.sync.dma_start(out=wqkv_sb[:, c * 4 * QKVW:(c + 1) * 4 * QKVW],
                                  in_=wqkv[:, c * 4 * QKVW:(c + 1) * 4 * QKVW])
            xb_tiles[0] = xb0
            nc.sync.dma_start(out=cos_sb[:, :], in_=cosb)
            nc.sync.dma_start(out=sin_sb[:, :], in_=sinb)

            def rope_and_v(sb, pga, pgb):
                rsrc = rt_p.tile([P, KW + QW], BF16, tag="rsrc", name="rsrc")
                nc.vector.tensor_copy(rsrc[:, 0:512], pga[:, :])
                nc.vector.tensor_copy(rsrc[:, 512:KW + QW], pgb[:, 0:128])
                nh = NKV + NQ
                rs = rsrc[:, :].rearrange("p (h i) -> p h i", h=nh)
                ev, od = rs[:, :, 0:32], rs[:, :, 32:64]
                c = cos_sb[:, sb * 32:(sb + 1) * 32].rearrange("p (o i) -> p o i", o=1).broadcast_to((P, nh, 32))
                s = sin_sb[:, sb * 32:(sb + 1) * 32].rearrange("p (o i) -> p o i", o=1).broadcast_to((P, nh, 32))
                t1 = rt_p.tile([P, nh * 32], BF16, tag="t1", name="t1")
                t2 = rt_p.tile([P, nh * 32], BF16, tag="t2", name="t2")
                t1r = t1[:, :].rearrange("p (h i) -> p h i", h=nh)
                t2r = t2[:, :].rearrange("p (h i) -> p h i", h=nh)
                dst = rot2[:, sb * 640:(sb + 1) * 640].rearrange("p (h i) -> p h i", h=nh)
                dst_e, dst_o = dst[:, :, 0:32], dst[:, :, 32:64]
                nc.vector.tensor_mul(t1r, ev, c)
                nc.vector.tensor_mul(t2r, od, s)
                nc.vector.tensor_sub(dst_e, t1r, t2r)
                nc.vector.tensor_mul(t1r, ev, s)
                nc.vector.tensor_mul(t2r, od, c)
                nc.vector.tensor_add(dst_o, t1r, t2r)
                vdst = vaug_sb[:, sb * 130: sb * 130 + 130].rearrange("p (kv c) -> p kv c", kv=2)[:, :, 0:D]
                vsrc = pgb[:, 128:256].rearrange("p (kv c) -> p kv c", kv=2)
                nc.vector.tensor_copy(vdst, vsrc)

            def wo_unit(pso_p, sb, n, copy_eng="v", tag="po"):
                po = pso_p.tile([P, 512], F32, tag=tag, name="po")
                for c in range(4):
                    nc.tensor.matmul(po[:, :], ctxT_sb[c][:, sb * P:(sb + 1) * P],
                                     wo_sb[:, c * HID + n * 512: c * HID + (n + 1) * 512],
                                     start=(c == 0), stop=(c == 3))
                ob = osb_p.tile([P, 512], BF16, tag="ob", name="ob")
                if copy_eng == "s":
                    nc.scalar.activation(ob[:, :], po[:, :], AF.Copy)
                else:
                    nc.vector.tensor_copy(ob[:, :], po[:, :])
                eng = nc.gpsimd if (sb + n) % 2 else nc.sync
                eng.dma_start(out=out[sb * P:(sb + 1) * P, n * 512:(n + 1) * 512],
                              in_=ob[:, :])

            def normalize(ctx, t, roff, qb):
                # copy ctx out of psum immediately (den + staged numerator)
                # so the pair's ctx psum frees after ~1us instead of after
                # the whole broadcast/recip/mul chain - the next unit's
                # first ctx matmul WAR-waits on this release
                den = nrm_p.tile([1, 512], F32, tag="den", name="den")
                nc.vector.tensor_copy(den[:, :], ctx[64:65, :])
                cstg = nrm_p.tile([D, 512], BF16, tag="cstg", name="cstg")
                nc.vector.tensor_copy(cstg[:, :], ctx[0:D, :])
                dbc = nrm_p.tile([D, 512], F32, tag="dbc", name="dbc")
                nc.gpsimd.partition_broadcast(dbc[:, :], den[:, :])
                rbc = nrm_p.tile([D, 512], F32, tag="rbc", name="rbc")
                nc.vector.reciprocal_approx_fast(out=rbc[:, :], in_=dbc[:, :])
                ntmp = nrm_p.tile([D, 512], BF16, tag="ntmp", name="ntmp")
                # qb=3 in two chunks: the tail wo units for sb 12,13 gate on
                # the first half of every head's last ctxT block
                cw = (256,) if qb == 3 else (512,)
                c0 = 0
                for w in (cw * 2 if qb == 3 else cw):
                    nc.vector.tensor_mul(ntmp[:, c0:c0 + w], cstg[:, c0:c0 + w], rbc[:, c0:c0 + w])
                    nc.sync.dma_start(
                        out=ctxT_sb[t][roff:roff + D, qb * 512 + c0:qb * 512 + c0 + w],
                        in_=ntmp[:, c0:c0 + w])
                    c0 += w

            def flip_closures(pt_pool, pt_tag, sb):
                # q/k flips: regular matmul with the block as the stationary
                # operand and identity as the moving one (out = blk.T @ I).
                def t_k():
                    pt = pt_pool.tile([P, P], F32, tag=pt_tag, name="pt")
                    nc.tensor.matmul(pt[:, :], rot2[:, sb * 640: sb * 640 + P],
                                     ident[:, :], start=True, stop=True)
                    nc.vector.tensor_copy(kT_sb[0][0:D, sb * P:(sb + 1) * P], pt[0:D, :])
                    nc.vector.tensor_copy(kT_sb[1][D:P, sb * P:(sb + 1) * P], pt[D:P, :])
                    nc.sync.dma_start(out=kT_sb[0][D:P, sb * P:(sb + 1) * P],
                                      in_=kT_sb[0][0:D, sb * P:(sb + 1) * P])
                    nc.sync.dma_start(out=kT_sb[1][0:D, sb * P:(sb + 1) * P],
                                      in_=kT_sb[1][D:P, sb * P:(sb + 1) * P])

                def t_q(t):
                    def f():
                        pt = pt_pool.tile([P, P], F32, tag=pt_tag, name="pt")
                        nc.tensor.matmul(pt[:, :],
                                         rot2[:, sb * 640 + KW + t * P: sb * 640 + KW + (t + 1) * P],
                                         ident[:, :], start=True, stop=True)
                        nc.vector.tensor_copy(qT_sb[t][:, sb * P:(sb + 1) * P], pt[:, :])
                    return f
                return [t_k] + [t_q(t) for t in range(4)]

            def proj_mms(psg_p, sb, kc_lo, kc_hi, tq, ti_box):
                """One piece of the qkv projection for seq block sb."""
                if kc_lo == 0:
                    if sb not in xb_tiles:
                        xb_load(sb)
                    if sb + 1 <= 15 and sb + 1 not in xb_tiles:
                        xb_load(sb + 1)   # prefetch next block
                    pga = psg_p.tile([P, 512], F32, tag="pga", name="pga")
                    pgb = psg_p.tile([P, 256], F32, tag="pgb", name="pgb")
                    xb_tiles[(sb, "ps")] = (pga, pgb)
                pga, pgb = xb_tiles[(sb, "ps")]
                xb = xb_tiles[sb]
                for kc in range(kc_lo, kc_hi):
                    nc.tensor.matmul(pga[:, :], xb[:, kc * P:(kc + 1) * P],
                                     wqkv_sb[:, kc * QKVW:kc * QKVW + 512],
                                     start=(kc == 0), stop=(kc == KC - 1))
                    nc.tensor.matmul(pgb[:, :], xb[:, kc * P:(kc + 1) * P],
                                     wqkv_sb[:, kc * QKVW + 512:(kc + 1) * QKVW],
                                     start=(kc == 0), stop=(kc == KC - 1))
                    if ti_box[0] < len(tq) and kc in (3, 6, 9, 12, 15):
                        tq[ti_box[0]]()
                        ti_box[0] += 1
                if kc_hi == KC:
                    while ti_box[0] < len(tq):
                        tq[ti_box[0]]()
                        ti_box[0] += 1
                    # stream in a quarter of wo during phase A (needed from
                    # A2 on; too big to load in one go at that point)
                    if sb in (2, 4, 6, 8):
                        c = (sb - 2) // 2
                        nc.sync.dma_start(out=wo_sb[:, c * HID:(c + 1) * HID],
                                          in_=wo[:, c * HID:(c + 1) * HID])
                    rope_and_v(sb, pga, pgb)
                    del xb_tiles[(sb, "ps")]
                    del xb_tiles[sb]

            def proj(psg_p, sb, tq=()):
                proj_mms(psg_p, sb, 0, KC, tq, [0])

            def emit_scores(ps_p, t, qb, kb):
                # head pair (2t, 2t+1): concurrent K=64 matmuls on row
                # groups (0,0)/(64,0); outputs in the two banks of one
                # [128,1024] f32 tile; one exp covers both halves (incl.
                # the stale gap cols on diagonal rounds - never read back)
                kv = t // 2
                j0 = max(kb - 4 * qb, 0)
                sT = ps_p.tile([P, 1024], F32, tag="sT", name="sT")
                nc.tensor.matmul(sT[:, j0 * P:512],
                                 kT_sb[kv][0:D, kb * P:(kb + 1) * P],
                                 qT_sb[t][0:D, qb * 512 + j0 * P:(qb + 1) * 512],
                                 start=True, stop=True)
                nc.tensor.matmul(sT[:, 512 + j0 * P:1024],
                                 kT_sb[kv][D:P, kb * P:(kb + 1) * P],
                                 qT_sb[t][D:P, qb * 512 + j0 * P:(qb + 1) * 512],
                                 start=True, stop=True)
                ex = exs_p.tile([P, 1024], BF16, tag="ex2", name="ex2")
                nc.scalar.activation(ex[:, j0 * P:1024], sT[:, j0 * P:1024],
                                     AF.Exp, scale=0.125)
                return ex

            def emit_ctx(ctxA, ctxB, t, qb, kb, nkb, ex):
                kv = t // 2
                j0 = max(kb - 4 * qb, 0)
                if kb >= 4 * qb:
                    nc.vector.tensor_mul(ex[:, j0 * P:(j0 + 1) * P],
                                         ex[:, j0 * P:(j0 + 1) * P], tri01[:, :])
                    nc.vector.tensor_mul(ex[:, 512 + j0 * P:512 + (j0 + 1) * P],
                                         ex[:, 512 + j0 * P:512 + (j0 + 1) * P], tri01[:, :])
                va = vaug_sb[:, kb * 130 + kv * 65: kb * 130 + (kv + 1) * 65]
                nc.tensor.matmul(ctxA[:, j0 * P:512], va, ex[:, j0 * P:512],
                                 start=(kb == 0), stop=(kb == nkb - 1),
                                 skip_group_check=True)
                nc.tensor.matmul(ctxB[:, j0 * P:512], va, ex[:, 512 + j0 * P:1024],
                                 start=(kb == 0), stop=(kb == nkb - 1),
                                 skip_group_check=True)
                if kb == nkb - 1:
                    normalize(ctxA, t, 0, qb)
                    normalize(ctxB, t, D, qb)

            # ======== phase A: proj 0..13 + attention groups 0,1 ========
            with tc.tile_pool(name="psg", bufs=2, space="PSUM") as psg_p, \
                 tc.tile_pool(name="pst", bufs=1, space="PSUM") as pst_p, \
                 tc.tile_pool(name="pssA", bufs=2, space="PSUM") as pssA_p, \
                 tc.tile_pool(name="pscA", bufs=1, space="PSUM") as pscA_p:

                def attn_unit_a(h, qb):
                    t, roff, kv = h // 2, D * (h % 2), h // 4
                    qT = qT_sb[t][roff:roff + D, :]
                    kT = kT_sb[kv][roff:roff + D, :]
                    ctx = pscA_p.tile([65, 512], F32, tag="ctx", name="ctx")
                    nkb = 4 * qb + 4
                    for kb in range(nkb):
                        j0 = max(kb - 4 * qb, 0)
                        sT = pssA_p.tile([P, 512], F32, tag="sT", name="sT")
                        nc.tensor.matmul(sT[:, j0 * P:512], kT[:, kb * P:(kb + 1) * P],
                                         qT[:, qb * 512 + j0 * P:(qb + 1) * 512], start=True, stop=True)
                        ex = exs_p.tile([P, 512], BF16, tag="ex", name="ex")
                        nc.scalar.activation(ex[:, j0 * P:512], sT[:, j0 * P:512], AF.Exp, scale=0.125)
                        if kb >= 4 * qb:
                            nc.vector.tensor_mul(ex[:, j0 * P:(j0 + 1) * P],
                                                 ex[:, j0 * P:(j0 + 1) * P], tri01[:, :])
                        nc.tensor.matmul(ctx[:, j0 * P:512],
                                         vaug_sb[:, kb * 130 + kv * 65: kb * 130 + (kv + 1) * 65],
                                         ex[:, j0 * P:512], start=(kb == 0), stop=(kb == nkb - 1),
                                         skip_group_check=True)
                    normalize(ctx, t, roff, qb)

                fa = lambda sb: flip_closures(pst_p, "pt", sb)
                proj(psg_p, 0)
                proj(psg_p, 1)
                for sb in (2, 3, 4, 5):
                    proj(psg_p, sb, fa(sb - 2))
                # group 0 (qb=0) with proj 6..9, two units per proj
                proj(psg_p, 6, fa(4))
                attn_unit_a(0, 0)
                attn_unit_a(1, 0)
                proj(psg_p, 7, fa(5))
                attn_unit_a(2, 0)
                attn_unit_a(3, 0)
                proj(psg_p, 8, fa(6))
                attn_unit_a(4, 0)
                attn_unit_a(5, 0)
                proj(psg_p, 9, fa(7))
                attn_unit_a(6, 0)
                attn_unit_a(7, 0)
                # group 1 (qb=1) with proj 10..13, two units per proj
                proj(psg_p, 10, fa(8))
                attn_unit_a(0, 1)
                attn_unit_a(1, 1)
                proj(psg_p, 11, fa(9))
                attn_unit_a(2, 1)
                attn_unit_a(3, 1)
                proj(psg_p, 12, fa(10))
                attn_unit_a(4, 1)
                wo_unit(pst_p, 0, 0, tag="pt")
                attn_unit_a(5, 1)
                wo_unit(pst_p, 0, 1, tag="pt")
                proj(psg_p, 13, fa(11))
                attn_unit_a(6, 1)
                wo_unit(pst_p, 0, 2, tag="pt")
                wo_unit(pst_p, 1, 0, tag="pt")
                attn_unit_a(7, 1)
                # wo for sb 0,1 needs only qb group 0; it fills the PE while
                # the last qb1 units' exp drains on ACT
                wo_unit(pst_p, 0, 3, tag="pt")
                wo_unit(pst_p, 1, 1, tag="pt")

            # ======== phase A2: qb group 2 pair rounds + proj 14,15 ========
            # + flips 12..15 + wo for seq blocks 0..3
            with tc.tile_pool(name="psg2", bufs=1, space="PSUM") as psg2_p, \
                 tc.tile_pool(name="ps2a", bufs=1, space="PSUM") as ps2a_p, \
                 tc.tile_pool(name="psca", bufs=1, space="PSUM") as psca_p, \
                 tc.tile_pool(name="psoa", bufs=2, space="PSUM") as psoa_p:

                # filler items, emitted between attention rounds
                items = []
                items.append(lambda: wo_unit(psoa_p, 1, 2))
                items.append(lambda: wo_unit(psoa_p, 1, 3))
                ti14, ti15 = [0], [0]
                tq14 = flip_closures(psoa_p, "po", 12)
                tq15 = flip_closures(psoa_p, "po", 13)
                for piece in range(4):
                    items.append(lambda p=piece: proj_mms(psg2_p, 14, p * 4, (p + 1) * 4, tq14, ti14))
                for piece in range(4):
                    items.append(lambda p=piece: proj_mms(psg2_p, 15, p * 4, (p + 1) * 4, tq15, ti15))
                for f in flip_closures(psoa_p, "po", 14):
                    items.append(f)
                for f in flip_closures(psoa_p, "po", 15):
                    items.append(f)
                for sb in (2, 3):
                    for n in range(4):
                        items.append(lambda sb=sb, n=n: wo_unit(psoa_p, sb, n))

                rounds = [(t, 2, kb, 12) for t in range(4) for kb in range(12)]
                ctx_tiles = {}

                def get_ctx(pool, kb):
                    if kb == 0:
                        ctx_tiles[0] = pool.tile([65, 512], F32, tag="ctxA", name="ctxA")
                        ctx_tiles[1] = pool.tile([65, 512], F32, tag="ctxB", name="ctxB")
                    return ctx_tiles[0], ctx_tiles[1]

                n_spread = len(items) - 8   # last 8 bridge the A2->B seam
                items[0]()   # 2 wo units keep the PE dense across the
                items[1]()   # phase A -> A2 pool transition
                ii = 2
                # per-round item quota: even spread plus a 2-item burst at
                # every unit start (the exp-gated first rounds starve the PE
                # and HAM re-throttles without dense filler there)
                quota = []
                q = 2.0
                per_round = (n_spread - 2 - 6) / len(rounds)
                for ri, (t, qb, kb, nkb) in enumerate(rounds):
                    if kb == 0 and ri > 0:
                        q += 2
                    q += per_round
                    quota.append(int(q))
                pend = None
                for ri, (t, qb, kb, nkb) in enumerate(rounds):
                    ex = emit_scores(ps2a_p, t, qb, kb)
                    while ii < n_spread and ii < quota[ri]:
                        items[ii]()
                        ii += 1
                    if pend is not None:
                        (pt_, pqb, pkb, pnkb, pex) = pend
                        ctxA, ctxB = get_ctx(psca_p, pkb)
                        emit_ctx(ctxA, ctxB, pt_, pqb, pkb, pnkb, pex)
                    pend = (t, qb, kb, nkb, ex)
                if pend is not None:
                    (pt_, pqb, pkb, pnkb, pex) = pend
                    ctxA, ctxB = get_ctx(psca_p, pkb)
                    emit_ctx(ctxA, ctxB, pt_, pqb, pkb, pnkb, pex)
                while ii < len(items):
                    items[ii]()
                    ii += 1

            # ======== phase B: qb group 3 pair rounds + wo 4..15 ========
            with tc.tile_pool(name="ps2", bufs=2, space="PSUM") as ps2_p, \
                 tc.tile_pool(name="psc", bufs=1, space="PSUM") as psc_p, \
                 tc.tile_pool(name="pso", bufs=2, space="PSUM") as pso_p:

                wos = [(sb, n) for sb in range(4, 16) for n in range(4)]
                wi = 0

                rounds = [(t, 3, kb, 16) for t in range(4) for kb in range(16)]
                ctx_tiles2 = {}

                def get_ctx2(kb):
                    if kb == 0:
                        ctx_tiles2[0] = psc_p.tile([65, 512], F32, tag="ctxA", name="ctxA")
                        ctx_tiles2[1] = psc_p.tile([65, 512], F32, tag="ctxB", name="ctxB")
                    return ctx_tiles2[0], ctx_tiles2[1]

                # software pipeline over round PAIRS: both rounds' score
                # pairs are emitted back-to-back (same row-group sequences
                # overlap their drains), then wo filler, then the previous
                # pair's ctx matmuls. wo cadence covers unit boundaries.
                pend = []
                oc = 0
                for r0 in range(0, len(rounds), 2):
                    chunk = rounds[r0:r0 + 2]
                    kb0 = chunk[0][2]
                    exs = [(t, qb, kb, nkb, emit_scores(ps2_p, t, qb, kb))
                           for (t, qb, kb, nkb) in chunk]
                    # guaranteed filler at unit-start chunks (kb 0,2), the
                    # exp-gated stretch where the PE otherwise starves
                    if kb0 in (0, 2):
                        if wi < 26:
                            wo_unit(pso_p, *wos[wi])
                            wi += 1
                    else:
                        if wi < 26 and oc % 4 < 3:
                            wo_unit(pso_p, *wos[wi])
                            wi += 1
                        oc += 1
                    for (pt_, pqb, pkb, pnkb, pex) in pend:
                        ctxA, ctxB = get_ctx2(pkb)
                        emit_ctx(ctxA, ctxB, pt_, pqb, pkb, pnkb, pex)
                    pend = exs
                for (pt_, pqb, pkb, pnkb, pex) in pend:
                    ctxA, ctxB = get_ctx2(pkb)
                    emit_ctx(ctxA, ctxB, pt_, pqb, pkb, pnkb, pex)
                # tail: the 2 held-back units bridge the normalize latency,
                # then sb 12..15 over rotated psum slots, copies alternating
                # between the idle scalar and vector engines
                tail_slots = [(pso_p, "po"), (psc_p, "ctxA"), (ps2_p, "sT"),
                              (pso_p, "po"), (psc_p, "ctxB"), (ps2_p, "sT")]
                for i, u in enumerate(wos[wi:]):
                    pool, tag = tail_slots[i % len(tail_slots)]
                    wo_unit(pool, *u, copy_eng=("s" if i % 2 else "v"), tag=tag)


def _build():
    nc = bacc.Bacc("TRN2", target_bir_lowering=False, debug=False, num_devices=8)
    # all inputs host-pre-permuted into DMA-friendly layouts (fat packets):
    # xT[sb*P+p, kc*P+j] = x.T[kc*P+p, sb*P+j]; weights/tables partition-major
    xT = nc.dram_tensor("xT", [S, HID], BF16, kind="ExternalInput").ap()
    wqkv = nc.dram_tensor("wqkv", [P, KC * QKVW], BF16, kind="ExternalInput").ap()
    wo = nc.dram_tensor("wo", [P, 4 * HID], BF16, kind="ExternalInput").ap()
    cosb = nc.dram_tensor("cosb", [P, SB * 32], BF16, kind="ExternalInput").ap()
    sinb = nc.dram_tensor("sinb", [P, SB * 32], BF16, kind="ExternalInput").ap()
    out = nc.dram_tensor("out", [S, HID], BF16, kind="ExternalOutput").ap()
    with tile.TileContext(nc) as tc:
        _emit_graph(nc, tc, xT, wqkv, wo, cosb, sinb, out)
    nc.finalize()
    return nc


def _planar(w, nheads):
    """Permute each head's 64 cols from interleaved (r0,i0,r1,i1,...) to
    planar (r0..r31, i0..i31)."""
    h = w.reshape(w.shape[0], nheads, 32, 2)
    return np.ascontiguousarray(h.transpose(0, 1, 3, 2).reshape(w.shape[0], nheads * 64))


def kernel(x, wq, wk, wv, wo, freqs_cos, freqs_sin, mask):
    x = np.asarray(x, dtype=np.float32)
    wq = np.asarray(wq, dtype=np.float32)
    wk = np.asarray(wk, dtype=np.float32)
    wv = np.asarray(wv, dtype=np.float32)
    wo = np.asarray(wo, dtype=np.float32)
    fc = np.asarray(freqs_cos, dtype=np.float32)
    fs = np.asarray(freqs_sin, dtype=np.float32)

    if "nc" not in _CACHE:
        _CACHE["nc"] = _build()
    nc = _CACHE["nc"]

    wqp = _planar(wq, 32)   # planar per q head
    wkp = _planar(wk, 8)    # planar per kv head
    cosb = np.ascontiguousarray(fc.reshape(SB, P, 32).transpose(1, 0, 2).reshape(P, SB * 32)).astype(BF)
    sinb = np.ascontiguousarray(fs.reshape(SB, P, 32).transpose(1, 0, 2).reshape(P, SB * 32)).astype(BF)
    in_maps = []
    for core in range(8):
        b, t = core // 4, core % 4
        # xT[sb*P+p, kc*P+j] = x[b][sb*P+j, kc*P+p]
        xTb = x[b].reshape(SB, P, KC, P).transpose(0, 3, 2, 1).reshape(S, HID)
        wqkv_full = np.concatenate(
            [wkp[:, t * KW:(t + 1) * KW],
             wqp[:, t * QW:(t + 1) * QW],
             wv[:, t * VW:(t + 1) * VW]], axis=1)   # [HID, 768]
        wqkv_p = wqkv_full.reshape(KC, P, QKVW).transpose(1, 0, 2).reshape(P, KC * QKVW)
        wo_p = wo[t * QW:(t + 1) * QW, :].reshape(4, P, HID).transpose(1, 0, 2).reshape(P, 4 * HID)
        in_maps.append({
            "xT": np.ascontiguousarray(xTb).astype(BF),
            "wqkv": np.ascontiguousarray(wqkv_p).astype(BF),
            "wo": np.ascontiguousarray(wo_p).astype(BF),
            "cosb": cosb, "sinb": sinb,
        })
    trace = bool(_CACHE.get("trace"))
    try:
        res = run_bass_kernel_spmd(nc, in_maps, list(range(8)), trace=trace)
    except Exception:
        if not trace:
            raise
        res = run_bass_kernel_spmd(nc, in_maps, list(range(8)))
    _CACHE["last_result"] = res
    outs = [np.asarray(r["out"], dtype=np.float32) for r in res.results]
    full = np.stack([outs[0] + outs[1] + outs[2] + outs[3],
                     outs[4] + outs[5] + outs[6] + outs[7]], axis=0)
    return full
